# revision 1
# baseline (speedup 1.0000x reference)
"""GAT (2-layer multi-head graph attention) on 8 Trainium2 NeuronCores.

Sharding: nodes (rows of adj / attention) are sharded across the 8 cores;
each core computes h = x@W replicated, its 512-row block of
e/softmax/aggregation for both GAT layers, with an AllGather of the layer-1
output (xcat) between layers.

Layout trick: attention probabilities are computed TRANSPOSED (eT[j, i]) so
softmax-normalizer and aggregation both run on the tensor engine:
  aggT[o, i] = sum_j hplus[j, o] * P[j, i]  with hplus = [h | 1] so the last
row of the accumulator is the softmax denominator Z.  exp/leaky run on the
scalar engine (Prelu alpha=0.2 + Exp share one ACT table set), masking is a
single DVE scalar_tensor_tensor using (adj-1)*100 added before the leaky
(masked entries land at ~exp(-16) -> 0).
"""
import os
import sys

for _p in ("/opt/trn_rl_repo", "/root/.axon_site/_ro/trn_rl_repo"):
    if os.path.isdir(_p) and _p not in sys.path:
        sys.path.insert(0, _p)

import numpy as np
import ml_dtypes

import concourse.bacc as bacc
import concourse.mybir as mybir
import concourse.tile as tile
from concourse import bass_utils

F32 = mybir.dt.float32
F32R = mybir.dt.float32r
BF16 = mybir.dt.bfloat16
AF = mybir.ActivationFunctionType
ALU = mybir.AluOpType

N, NFEAT, NHID, NCLASS, NHEADS = 4096, 512, 64, 128, 8
NCORES = 8
R = N // NCORES          # 512 rows per core
FC = NFEAT // 128        # 4 feature chunks
JC = N // 128            # 32 j-chunks
BIG = 100.0
ALPHA = 0.2

_CACHE = {}


def _build_nc():
    nc = bacc.Bacc("TRN2", target_bir_lowering=False, debug=False,
                   num_devices=NCORES)

    xT_d = nc.dram_tensor("xT", [NFEAT, N], F32R, kind="ExternalInput")
    xTb_d = nc.dram_tensor("xTblk", [NFEAT, R], F32R, kind="ExternalInput")
    Wcat_d = nc.dram_tensor("Wcat", [NFEAT, 512], F32R, kind="ExternalInput")
    WcatT_d = nc.dram_tensor("WcatT", [512, NFEAT], F32R, kind="ExternalInput")
    A12_d = nc.dram_tensor("A12", [512, 16], F32R, kind="ExternalInput")
    Wout_d = nc.dram_tensor("Wout", [512, NCLASS], F32R, kind="ExternalInput")
    WoutT_d = nc.dram_tensor("WoutT", [NCLASS, 512], F32R, kind="ExternalInput")
    AO_d = nc.dram_tensor("AO", [NCLASS, 2], F32R, kind="ExternalInput")
    adj_d = nc.dram_tensor("adjm1T", [N, R], BF16, kind="ExternalInput")
    id_d = nc.dram_tensor("ident", [128, 128], F32, kind="ExternalInput")
    out_d = nc.dram_tensor("out", [R, NCLASS], F32, kind="ExternalOutput")

    with tile.TileContext(nc, num_cores=NCORES) as tc:
        with (
            tc.tile_pool(name="persist", bufs=1) as Pp,
            tc.tile_pool(name="dram", bufs=1, space="DRAM") as Pd,
            tc.tile_pool(name="psA", bufs=2, space="PSUM") as PsA,
            tc.tile_pool(name="psS", bufs=2, space="PSUM") as PsS,
            tc.tile_pool(name="pagg", bufs=1, space="PSUM") as Pagg,
        ):
            # ---- persistent constants / small state ----
            alpha = Pp.tile([128, 1], F32, name="alpha")
            nc.vector.memset(alpha[:], ALPHA)
            onescol = Pp.tile([128, 1], F32R, name="onescol")
            nc.vector.memset(onescol[:].bitcast(F32), 1.0)
            sfjT = Pp.tile([128, JC, 8], F32, name="sfjT")
            sxcb = Pp.tile([128, FC, R], F32, name="sxcb")  # own xcatT block
            sw12 = Pp.tile([128, FC, 16], F32, name="sw12")
            sWcatF = Pp.tile([128, FC, 512], F32, name="sWcatF")
            for fc in range(FC):
                nc.sync.dma_start(
                    sWcatF[:, fc, :],
                    Wcat_d.ap()[fc * 128:(fc + 1) * 128, :].bitcast(F32))
            sWout = Pp.tile([128, FC, NCLASS], F32, name="sWout")
            for fc in range(FC):
                nc.sync.dma_start(
                    sWout[:, fc, :],
                    Wout_d.ap()[fc * 128:(fc + 1) * 128, :].bitcast(F32))
            sWoutT = Pp.tile([128, 512], F32, name="sWoutT")
            nc.sync.dma_start(sWoutT[:], WoutT_d.ap().bitcast(F32))
            sAO = Pp.tile([128, 2], F32, name="sAO")
            nc.sync.dma_start(sAO[:], AO_d.ap().bitcast(F32))
            sw2 = Pp.tile([128, FC, 2], F32, name="sw2")
            for fc in range(FC):
                pw2 = PsS.tile([128, 2], F32, tag="ps_s", bufs=2)
                nc.tensor.matmul(
                    pw2[:], sWoutT[:, fc * 128:(fc + 1) * 128], sAO[:],
                    start=True, stop=True)
                nc.vector.tensor_copy(sw2[:, fc, :], pw2[:])
            fibcat = Pp.tile([128, NHEADS * R], F32, name="fibcat")

            with tc.tile_pool(name="hplusp", bufs=1) as Ph:
                shplus = Ph.tile([128, JC, NHEADS, NHID + 1], F32R, name="shplus")
                nc.vector.memset(shplus[:, :, :, NHID].bitcast(F32), 1.0)

                # ================= stage 1: weights / fifj =================
                with tc.tile_pool(name="stage1", bufs=1) as P1:
                    sfown = P1.tile([16, R], F32, name="sfown")

                    with tc.tile_pool(name="stage1a", bufs=1) as P1a:
                        sA12 = P1a.tile([128, 4, 16], F32, name="sA12")
                        for hoc in range(4):
                            nc.sync.dma_start(
                                sA12[:, hoc, :],
                                A12_d.ap()[hoc * 128:(hoc + 1) * 128, :].bitcast(F32))
                        sxTb = P1a.tile([128, FC, R], F32, name="sxTb")
                        for fc in range(FC):
                            nc.sync.dma_start(
                                sxTb[:, fc, :],
                                xTb_d.ap()[fc * 128:(fc + 1) * 128, :].bitcast(F32))

                        # w12[f, k] = sum_ho WcatT[ho, f] * A12[ho, k]
                        # 4 parallel slab DMAs up front, then back-to-back mms
                        sWcT = P1a.tile([128, 4, NFEAT], F32, name="sWcT")
                        for hoc in range(4):
                            nc.sync.dma_start(
                                sWcT[:, hoc, :],
                                WcatT_d.ap()[hoc * 128:(hoc + 1) * 128, :]
                                .bitcast(F32))
                        for fc in range(FC):
                            pw = PsS.tile([128, 16], F32, tag="ps_s", bufs=2)
                            for hoc in range(4):
                                nc.tensor.matmul(
                                    pw[:],
                                    sWcT[:, hoc, fc * 128:(fc + 1) * 128],
                                    sA12[:, hoc, :],
                                    start=(hoc == 0), stop=(hoc == 3))
                            nc.vector.tensor_copy(sw12[:, fc, :], pw[:])

                        def prep_jc(jc):
                            """stage-A hplus[jc] + fj columns[jc], exact fp32,
                            streaming x tiles from DRAM."""
                            xa = []
                            for fc in range(FC):
                                t = Pp.tile([128, 128], F32, tag=f"xa{fc}",
                                            bufs=2, name=f"xa{fc}_{jc}")
                                nc.sync.dma_start(
                                    t[:], xT_d.ap()[fc * 128:(fc + 1) * 128,
                                                    jc * 128:(jc + 1) * 128]
                                    .bitcast(F32))
                                xa.append(t)
                            pA = PsA.tile([128, 512], F32, tag="ps_a", bufs=2,
                                          name=f"pA{jc}")
                            for fc in range(FC):
                                nc.tensor.matmul(
                                    pA[:], xa[fc][:], sWcatF[:, fc, :],
                                    start=(fc == 0), stop=(fc == 3))
                            nc.vector.tensor_copy(
                                shplus[:, jc, :, 0:NHID],
                                pA[:].rearrange("p (hd o) -> p hd o", o=NHID))
                            pfj = PsS.tile([128, 8], F32, tag="ps_s", bufs=2,
                                           name=f"pfj{jc}")
                            for fc in range(FC):
                                nc.tensor.matmul(
                                    pfj[:], xa[fc][:], sw12[:, fc, 0:8],
                                    start=(fc == 0), stop=(fc == 3))
                            nc.vector.tensor_copy(sfjT[:, jc, :], pfj[:])


                        prep_jc(0)
                        prep_jc(1)

                        # own-block fifj (for fi of this core's rows)
                        pfo = PsS.tile([16, 512], F32, tag="ps_s", bufs=2)
                        for fc in range(FC):
                            nc.tensor.matmul(
                                pfo[:], sw12[:, fc, :], sxTb[:, fc, :],
                                start=(fc == 0), stop=(fc == 3))
                        nc.vector.tensor_copy(sfown[:], pfo[:])

                    # all 8 fi rows -> one [1, 8*R] row, one broadcast;
                    # fib[hd] is then a free-dim slice of fibcat
                    fcat = P1.tile([1, NHEADS * R], F32, name="fcat")
                    nc.gpsimd.dma_start(
                        fcat[:].rearrange("o (hd r) -> o hd r", hd=NHEADS),
                        sfown[8:16, :].rearrange("hd r -> () hd r")
                        if False else sfown[8:16, :])
                    nc.gpsimd.partition_broadcast(fibcat[:], fcat[:])

                # ================= layer-1 attention sweeps =================
                with tc.tile_pool(name="chunkL1", bufs=1) as Pc:
                    paggs = {}
                    for sweep in range(2):
                        heads = list(range(sweep * 4, sweep * 4 + 4))
                        for jc in range(JC):
                            if sweep == 0 and jc + 2 < JC:
                                prep_jc(jc + 2)
                            mask = Pc.tile([128, 512], BF16, tag="mask", bufs=3)
                            nc.sync.dma_start(
                                mask[:], adj_d.ap()[jc * 128:(jc + 1) * 128, :])
                            raw4 = Pc.tile([128, 2048], F32, tag="raw4", bufs=2)
                            em4 = Pc.tile([128, 2048], F32, tag="em4", bufs=3)
                            P4 = Pc.tile([128, 2048], F32R, tag="p4", bufs=2)
                            for q, hd in enumerate(heads):
                                sl = slice(q * 512, (q + 1) * 512)
                                gidx = (sweep * JC + jc) * 4 + q
                                nc.vector.scalar_tensor_tensor(
                                    raw4[:, sl], mask[:], BIG,
                                    fibcat[:, hd * R:(hd + 1) * R],
                                    op0=ALU.mult, op1=ALU.add)
                                if (gidx * 7) % 26 < 7:
                                    u = Pc.tile([128, 512], F32, tag="ulk",
                                                bufs=3)
                                    nc.vector.tensor_scalar_add(
                                        u[:], raw4[:, sl],
                                        sfjT[:, jc, hd:hd + 1])
                                    nc.vector.scalar_tensor_tensor(
                                        em4[:, sl], u[:], ALPHA, u[:],
                                        op0=ALU.mult, op1=ALU.max)
                                else:
                                    nc.scalar.activation(
                                        em4[:, sl], raw4[:, sl], AF.Prelu,
                                        bias=sfjT[:, jc, hd:hd + 1],
                                        alpha=alpha[:])
                            nc.scalar.activation(P4[:], em4[:], AF.Exp)
                            for q, hd in enumerate(heads):
                                if jc == 0:
                                    paggs[hd] = Pagg.tile(
                                        [NHID + 1, 512], F32, tag=f"agg{q}",
                                        bufs=1, name=f"agg_s{sweep}_{q}")
                                nc.tensor.matmul(
                                    paggs[hd][:], shplus[:, jc, hd, :],
                                    P4[:, q * 512:(q + 1) * 512],
                                    start=(jc == 0), stop=(jc == JC - 1))

                        # normalize this sweep's heads into the xcatT block
                        zsw = Pc.tile([4, R], F32, tag="zsw", bufs=2)
                        for q, hd in enumerate(heads):
                            zst = Pc.tile([NHID + 1, R], F32, tag="zst", bufs=2)
                            nc.vector.tensor_copy(
                                zst[NHID:NHID + 1, :], paggs[hd][NHID:NHID + 1, :])
                            nc.gpsimd.dma_start(
                                zsw[q:q + 1, :], zst[NHID:NHID + 1, :])
                        rzw = Pc.tile([4, R], F32, tag="rzw", bufs=2)
                        rzs = Pc.tile([4, R], F32, tag="rzs", bufs=2)
                        nc.vector.reciprocal_approx_accurate(
                            rzw[:], zsw[:], rzs[:])
                        for q, hd in enumerate(heads):
                            rzt = Pc.tile([1, R], F32, tag="rzt", bufs=2)
                            nc.gpsimd.dma_start(rzt[:], rzw[q:q + 1, :])
                            zb = Pc.tile([64, R], F32, tag="zb", bufs=2)
                            nc.gpsimd.partition_broadcast(zb[:], rzt[:])
                            xcn = Pc.tile([64, R], F32, tag="xcn", bufs=2)
                            nc.vector.tensor_mul(
                                xcn[:], paggs[hd][0:NHID, :], zb[:])
                            nc.gpsimd.dma_start(
                                sxcb[64 * (hd % 2):64 * (hd % 2) + 64,
                                     hd // 2, :], xcn[:])

            # ===== layer-2 projections on the OWN block, then small gather =====
            # h2_block[n, c] = sum_f xcat_blk[n, f] Wout[f, c]   (own 512 nodes)
            # fifj2_block = w2.T @ xcat_blkT  -> fi2 (row 0, local), fj2 (row 1)
            dblk2 = Pd.tile([R, NCLASS], F32, name="dblk2")
            dgath2 = Pd.tile([N, NCLASS], F32, name="dgath2",
                             addr_space="Shared")
            dblk2b = Pd.tile([1, R], F32, name="dblk2b")
            dgath2b = Pd.tile([8, R], F32, name="dgath2b",
                              addr_space="Shared")
            sfo2 = Pp.tile([2, R], F32, name="sfo2")
            pf2o = PsS.tile([2, 512], F32, tag="ps_s", bufs=2)
            for fc in range(FC):
                nc.tensor.matmul(
                    pf2o[:], sw2[:, fc, :], sxcb[:, fc, :],
                    start=(fc == 0), stop=(fc == 3))
            nc.vector.tensor_copy(sfo2[:], pf2o[:])
            nc.gpsimd.dma_start(dblk2b[:], sfo2[1:2, :])
            for nc4 in range(4):
                pH = PsA.tile([128, 512], F32, tag="ps_a", bufs=2)
                for fc in range(FC):
                    nc.tensor.matmul(
                        pH[:, 0:NCLASS],
                        sxcb[:, fc, nc4 * 128:(nc4 + 1) * 128],
                        sWout[:, fc, :],
                        start=(fc == 0), stop=(fc == 3))
                sh2b = Pp.tile([128, NCLASS], F32, tag="sh2b", bufs=2)
                nc.vector.tensor_copy(sh2b[:], pH[:, 0:NCLASS])
                nc.gpsimd.dma_start(
                    dblk2[nc4 * 128:(nc4 + 1) * 128, :], sh2b[:])
            nc.gpsimd.collective_compute(
                "AllGather", ALU.bypass,
                replica_groups=[list(range(NCORES))],
                ins=[dblk2b[:].opt()], outs=[dgath2b[:].opt()])
            nc.gpsimd.collective_compute(
                "AllGather", ALU.bypass,
                replica_groups=[list(range(NCORES))],
                ins=[dblk2[:].opt()], outs=[dgath2[:].opt()])

            # ======================== layer 2 ========================
            with tc.tile_pool(name="stage2", bufs=1) as P2:
                sfj2T = P2.tile([128, JC], F32, name="sfj2T")
                nc.gpsimd.dma_start(
                    sfj2T[:].rearrange("p (r jc) -> p r jc", r=8),
                    dgath2b[:].rearrange("r (jc p) -> p r jc", p=128))
                fib2 = P2.tile([128, R], F32, name="fib2")
                nc.gpsimd.partition_broadcast(fib2[:], sfo2[0:1, :])
                sh2r = P2.tile([128, JC, NCLASS], F32R, name="sh2r")
                for jc in range(JC):
                    nc.gpsimd.dma_start(
                        sh2r[:, jc, :],
                        dgath2[jc * 128:(jc + 1) * 128, :].bitcast(F32R))
                ident = P2.tile([128, 128], F32, name="ident")
                nc.sync.dma_start(ident[:], id_d.ap())


                # layer-2 attention chunks (batch 4 jc per Exp)
                pagg2 = Pagg.tile([128, 512], F32, tag="agg0", bufs=1)
                pZ2 = Pagg.tile([1, 512], F32, tag="agg1", bufs=1)
                for jb in range(8):
                    raw4 = P2.tile([128, 2048], F32, tag="raw4b", bufs=3)
                    em4 = P2.tile([128, 2048], F32, tag="em4b", bufs=3)
                    P4 = P2.tile([128, 2048], F32R, tag="p4b", bufs=7)
                    for q in range(4):
                        jc = jb * 4 + q
                        sl = slice(q * 512, (q + 1) * 512)
                        mask = P2.tile([128, 512], BF16, tag="maskb", bufs=3)
                        nc.sync.dma_start(
                            mask[:], adj_d.ap()[jc * 128:(jc + 1) * 128, :])
                        nc.vector.scalar_tensor_tensor(
                            raw4[:, sl], mask[:], BIG, fib2[:],
                            op0=ALU.mult, op1=ALU.add)
                        if (jc * 7) % 26 < 7:
                            u = P2.tile([128, 512], F32, tag="ulk2", bufs=3)
                            nc.vector.tensor_scalar_add(
                                u[:], raw4[:, sl], sfj2T[:, jc:jc + 1])
                            nc.vector.scalar_tensor_tensor(
                                em4[:, sl], u[:], ALPHA, u[:],
                                op0=ALU.mult, op1=ALU.max)
                        else:
                            nc.scalar.activation(
                                em4[:, sl], raw4[:, sl], AF.Prelu,
                                bias=sfj2T[:, jc:jc + 1], alpha=alpha[:])
                    nc.scalar.activation(P4[:], em4[:], AF.Exp)
                    for q in range(4):
                        jc = jb * 4 + q
                        sl = slice(q * 512, (q + 1) * 512)
                        nc.tensor.matmul(
                            pagg2[:], sh2r[:, jc, :], P4[:, sl],
                            start=(jc == 0), stop=(jc == JC - 1))
                        nc.tensor.matmul(
                            pZ2[:], onescol[:], P4[:, sl],
                            start=(jc == 0), stop=(jc == JC - 1))

                # normalize, elu (per 64-class half), then transpose
                sz2 = P2.tile([1, R], F32, name="sz2")
                nc.vector.tensor_copy(sz2[:], pZ2[0:1, :])
                srz2 = P2.tile([1, R], F32, name="srz2")
                srz2s = P2.tile([1, R], F32, name="srz2s")
                nc.vector.reciprocal_approx_accurate(
                    srz2[:], sz2[:], srz2s[:])
                zb2 = P2.tile([64, R], F32, name="zb2")
                nc.gpsimd.partition_broadcast(zb2[:], srz2[:], channels=64)
                halves = []
                for nmh, pg in (("a", pagg2[0:64, :]), ("c", pagg2[64:128, :])):
                    sv = P2.tile([64, R], F32, tag="sv", bufs=1,
                                 name=f"sv{nmh}")
                    nc.vector.tensor_mul(sv[:], pg, zb2[:])
                    smin = P2.tile([64, R], F32, tag="smin", bufs=1,
                                   name=f"smin{nmh}")
                    nc.vector.tensor_scalar_min(smin[:], sv[:], 0.0)
                    sex = P2.tile([64, R], F32, tag="sex", bufs=1,
                                  name=f"sex{nmh}")
                    nc.scalar.activation(sex[:], smin[:], AF.Exp)
                    srel = P2.tile([64, R], F32, tag="srel", bufs=1,
                                   name=f"srel{nmh}")
                    nc.scalar.activation(srel[:], sv[:], AF.Relu)
                    sres = P2.tile([64, R], F32, tag=f"sres{nmh}", bufs=1,
                                   name=f"sres{nmh}")
                    nc.vector.scalar_tensor_tensor(
                        sres[:], sex[:], -1.0, srel[:],
                        op0=ALU.add, op1=ALU.add)
                    halves.append(sres)

                sts, negmxs, ssums = [], [], []
                for it in range(4):
                    st = P2.tile([128, 128], F32, tag="st", bufs=4,
                                 name=f"st{it}")
                    for q, sres in enumerate(halves):
                        ptp = PsS.tile([128, 64], F32, tag="ps_s", bufs=2,
                                       name=f"ptp{it}_{q}")
                        nc.tensor.transpose(
                            ptp[:], sres[:, it * 128:(it + 1) * 128],
                            ident[0:64, 0:64])
                        nc.vector.tensor_copy(
                            st[:, q * 64:(q + 1) * 64], ptp[:])
                    mx = P2.tile([128, 1], F32, tag="mx", bufs=4,
                                 name=f"mx{it}")
                    nc.vector.tensor_reduce(
                        mx[:], st[:], axis=mybir.AxisListType.X, op=ALU.max)
                    negmx = P2.tile([128, 1], F32, tag="negmx", bufs=4,
                                    name=f"negmx{it}")
                    nc.vector.tensor_scalar_mul(negmx[:], mx[:], -1.0)
                    sts.append(st); negmxs.append(negmx)
                for it in range(4):
                    sexp = P2.tile([128, 128], F32, tag="sexp", bufs=2,
                                   name=f"sexp{it}")
                    ssum = P2.tile([128, 1], F32, tag="ssum", bufs=4,
                                   name=f"ssum{it}")
                    nc.scalar.activation(
                        sexp[:], sts[it][:], AF.Exp, bias=negmxs[it][:],
                        accum_out=ssum[:])
                    ssums.append(ssum)
                slns = []
                for it in range(4):
                    sln = P2.tile([128, 1], F32, tag="sln", bufs=4,
                                  name=f"sln{it}")
                    nc.scalar.activation(sln[:], ssums[it][:], AF.Ln)
                    slns.append(sln)
                for it in range(4):
                    b2 = P2.tile([128, 1], F32, tag="b2", bufs=4,
                                 name=f"b2{it}")
                    nc.vector.tensor_sub(b2[:], negmxs[it][:], slns[it][:])
                    sout = P2.tile([128, 128], F32, tag="sout", bufs=2,
                                   name=f"sout{it}")
                    nc.scalar.activation(sout[:], sts[it][:], AF.Identity,
                                         bias=b2[:])
                    nc.sync.dma_start(
                        out_d.ap()[it * 128:(it + 1) * 128, :], sout[:])

    nc.finalize()
    return nc


def _get_nc():
    if "nc" not in _CACHE:
        _CACHE["nc"] = _build_nc()
    return _CACHE["nc"]


def kernel(**inputs):
    x = np.asarray(inputs["x"], dtype=np.float32)
    adj = np.asarray(inputs["adj"])
    W = np.asarray(inputs["W"], dtype=np.float32)
    a = np.asarray(inputs["a"], dtype=np.float32)
    W_out = np.asarray(inputs["W_out"], dtype=np.float32)
    a_out = np.asarray(inputs["a_out"], dtype=np.float32)

    xT = np.ascontiguousarray(x.T)
    Wcat = np.ascontiguousarray(W.transpose(1, 0, 2).reshape(NFEAT, 512))
    WcatT = np.ascontiguousarray(Wcat.T)
    A12 = np.zeros((512, 16), np.float32)
    for hd in range(NHEADS):
        A12[hd * NHID:(hd + 1) * NHID, hd] = a[hd, NHID:]      # a2 -> fj
        A12[hd * NHID:(hd + 1) * NHID, 8 + hd] = a[hd, :NHID]  # a1 -> fi
    WoutT = np.ascontiguousarray(W_out.T)
    AO = np.stack([a_out[:NCLASS], a_out[NCLASS:]], axis=1)
    AO = np.ascontiguousarray(AO, dtype=np.float32)
    ident = np.eye(128, dtype=np.float32)
    adjm1 = adj.astype(np.float32) - 1.0

    in_maps = []
    for c in range(NCORES):
        r0, r1 = c * R, (c + 1) * R
        in_maps.append({
            "xT": xT,
            "xTblk": np.ascontiguousarray(x[r0:r1].T),
            "Wcat": Wcat,
            "WcatT": WcatT,
            "A12": A12,
            "Wout": W_out,
            "WoutT": WoutT,
            "AO": AO,
            "adjm1T": np.ascontiguousarray(adjm1[r0:r1].T).astype(
                ml_dtypes.bfloat16),
            "ident": ident,
        })

    nc = _get_nc()
    trace = bool(os.environ.get("KERNEL_TRACE"))
    res = bass_utils.run_bass_kernel_spmd(
        nc, in_maps, list(range(NCORES)), trace=trace)
    kernel.last_results = res
    out = np.concatenate(
        [res.results[c]["out"] for c in range(NCORES)], axis=0)
    return np.ascontiguousarray(out, dtype=np.float32)



# revision 50
# speedup vs baseline: 1.5916x; 1.5916x over previous
"""GAT (2-layer multi-head graph attention) on 8 Trainium2 NeuronCores.

Sharding: nodes (rows of adj / attention) are sharded across the 8 cores;
each core computes h = x@W replicated, its 512-row block of
e/softmax/aggregation for both GAT layers.  The layer-1 -> layer-2
projections (h2|fj2|fi2 packed as 130 cols) are AllGathered in TWO halves:
the heads-0..3 partial right after sweep 0 (hidden under sweep 1), the
heads-4..7 partial at the end; layer 2 sums the gathered partials.

Layout: attention probabilities are computed TRANSPOSED (eT[j, i]) so the
softmax-normalizer and aggregation both run on the tensor engine via an
hplus = [h | 1] stationary operand (last row of the accumulator is Z).

Key factorization: softmax over j is invariant to any per-i shift, so we
compute e'[j,i] = leakyrelu(fi[i]+fj[j]) - fi[i] = max(fj, 0.2*fj - 0.8*fi).
With fib08 = -0.8*fi broadcast tiles precomputed once per head, the whole
e-map is ONE tensor_scalar (DVE 4x mode) per tile plus exp plus mask:
  em = (fib08 + 0.2*fj) max fj  (tensor_scalar, two ptr scalars, DVE 4x)
  P  = exp(em)                  (scalar engine; one [128,4096] op per 2 jc)
  Pm = P * adj                  (tensor_tensor 2x, split 6:2 DVE/Pool)
The 0/1 adjacency multiply replaces the -1e9 mask (exact zeros).
Weight-only transforms (Wcat, w12 = Wcat @ A12, w2 appended to Wout) are
folded on the host.  Z-reciprocal broadcasts are built with one-hot outer
products on the tensor engine.  Idle-window dummy matmuls keep the PE
p-state warm across the exposed collective.
"""
import os
import sys

for _p in ("/opt/trn_rl_repo", "/root/.axon_site/_ro/trn_rl_repo"):
    if os.path.isdir(_p) and _p not in sys.path:
        sys.path.insert(0, _p)

import numpy as np
import ml_dtypes

import concourse.bacc as bacc
import concourse.mybir as mybir
import concourse.tile as tile
from concourse import bass_utils

F32 = mybir.dt.float32
F32R = mybir.dt.float32r
BF16 = mybir.dt.bfloat16
AF = mybir.ActivationFunctionType
ALU = mybir.AluOpType

N, NFEAT, NHID, NCLASS, NHEADS = 4096, 512, 64, 128, 8
NCORES = 8
R = N // NCORES          # 512 rows per core
FC = NFEAT // 128        # 4 feature chunks
JC = N // 128            # 32 j-chunks
NP = JC // 2             # 16 jc-pairs
NP_BUILD = int(os.environ.get('NP_BUILD', '0'))
ALPHA = 0.2

_CACHE = {}
SKIP_WARM = bool(os.environ.get('SKIP_WARM'))
SKIP_L2 = bool(os.environ.get('SKIP_L2'))
SKIP_EVAC_ACT = bool(os.environ.get('SKIP_EVAC_ACT'))


def _build_nc():
    nc = bacc.Bacc("TRN2", target_bir_lowering=False, debug=False,
                   num_devices=NCORES)

    xT_d = nc.dram_tensor("xT", [NFEAT, N], BF16, kind="ExternalInput")
    xTb_d = nc.dram_tensor("xTblk", [NFEAT, R], BF16, kind="ExternalInput")
    Wcat_d = nc.dram_tensor("Wcat", [NFEAT, 512], BF16, kind="ExternalInput")
    W12_d = nc.dram_tensor("W12", [NFEAT, 16], BF16, kind="ExternalInput")
    WoutP_d = nc.dram_tensor("WoutP", [512, 130], BF16, kind="ExternalInput")
    adj_d = nc.dram_tensor("adjT", [N, R], BF16, kind="ExternalInput")
    sel_d = nc.dram_tensor("sel4", [4, 512], BF16, kind="ExternalInput")
    id_d = nc.dram_tensor("ident", [128, 128], F32, kind="ExternalInput")
    out_d = nc.dram_tensor("out", [R, NCLASS], F32, kind="ExternalOutput")

    with tile.TileContext(nc, num_cores=NCORES) as tc:
        with (
            tc.tile_pool(name="persist", bufs=1) as Pp,
            tc.tile_pool(name="dram", bufs=1, space="DRAM") as Pd,
            tc.tile_pool(name="psA", bufs=2, space="PSUM") as PsA,
            tc.tile_pool(name="psS", bufs=2, space="PSUM") as PsS,
            tc.tile_pool(name="pagg", bufs=1, space="PSUM") as Pagg,
        ):
            # ---- persistent constants ----
            onescol = Pp.tile([128, 1], BF16, name="onescol")
            nc.vector.memset(onescol[:], 1.0)
            ones1 = Pp.tile([1, 128], BF16, name="ones1")
            nc.vector.memset(ones1[:], 1.0)
            sel = Pp.tile([4, 512], BF16, name="sel")
            nc.sync.dma_start(sel[:], sel_d.ap())

            # ---- DMAs ordered so the L1 pipeline can start ASAP ----
            sWcatF = Pp.tile([128, FC, 512], BF16, name="sWcatF")
            nc.sync.dma_start(
                sWcatF[:],
                Wcat_d.ap().rearrange("(fc p) o -> p fc o", p=128))
            sw12 = Pp.tile([128, FC, 16], BF16, name="sw12")
            nc.sync.dma_start(
                sw12[:], W12_d.ap().rearrange("(fc p) o -> p fc o", p=128))
            sxTb = Pp.tile([128, FC, R], BF16, name="sxTb")
            nc.sync.dma_start(
                sxTb[:], xTb_d.ap().rearrange("(fc p) r -> p fc r", p=128))
            ident = Pp.tile([128, 128], F32R, name="ident")
            nc.sync.dma_start(ident[:], id_d.ap())

            # x and adjacency stream in j-order, interleaved
            sx = Pp.tile([128, FC, N], BF16, name="sx")
            masks = Pp.tile([128, JC, R], BF16, name="masks")
            for g in range(4):
                nc.sync.dma_start(
                    sx[:, :, g * 1024:(g + 1) * 1024],
                    xT_d.ap()[:, g * 1024:(g + 1) * 1024]
                    .rearrange("(fc p) j -> p fc j", p=128))
                nc.sync.dma_start(
                    masks[:, g * 8:(g + 1) * 8, :],
                    adj_d.ap()[g * 1024:(g + 1) * 1024, :]
                    .rearrange("(jc p) r -> p jc r", p=128))
            sWoutP = Pp.tile([64, NHEADS, 130], BF16, name="sWoutP")
            nc.sync.dma_start(
                sWoutP[:], WoutP_d.ap().rearrange("(hd p) c -> p hd c", p=64))

            sfjT = Pp.tile([128, JC, 8], F32, name="sfjT")
            sfjT2 = Pp.tile([128, JC, 8], F32, name="sfjT2")  # 0.2 * fj
            fib08 = Pp.tile([128, NHEADS * R], BF16, name="fib08")  # -0.8*fi
            sxc = Pp.tile([64, NHEADS, R], BF16, name="sxc")
            szall = Pp.tile([8, R], F32, name="szall")

            shplus = Pp.tile([128, JC, NHEADS, NHID + 1], BF16, name="shplus")
            nc.vector.memset(shplus[:, :, :, NHID], 1.0)

            def warm_pe(n, tag, shape_src):
                """Dummy matmuls that keep the PE p-state ramped while it
                would otherwise idle (in-order PE queue = placement)."""
                w = Pagg.tile([1, 512], F32, tag=tag, bufs=1, name=f"w{tag}")
                for _ in range(n):
                    nc.tensor.matmul(w[:], onescol[:], shape_src,
                                     start=True, stop=True)

            # ================= prep: h tiles / fj / fi =================
            def prep_jc(jc):
                xa = [sx[:, fc, jc * 128:(jc + 1) * 128] for fc in range(FC)]
                pA = PsA.tile([128, 512], F32, tag="ps_a", bufs=2,
                              name=f"pA{jc}")
                for fc in range(FC):
                    nc.tensor.matmul(
                        pA[:], xa[fc], sWcatF[:, fc, :],
                        start=(fc == 0), stop=(fc == 3))
                nc.gpsimd.tensor_copy(
                    shplus[:, jc, :, 0:NHID],
                    pA[:].rearrange("p (hd o) -> p hd o", o=NHID))
                pfj = PsS.tile([128, 8], F32, tag="ps_s", bufs=2,
                               name=f"pfj{jc}")
                for fc in range(FC):
                    nc.tensor.matmul(
                        pfj[:], xa[fc], sw12[:, fc, 0:8],
                        start=(fc == 0), stop=(fc == 3))
                nc.gpsimd.tensor_copy(sfjT[:, jc, :], pfj[:])
                nc.gpsimd.tensor_scalar_mul(sfjT2[:, jc, :], pfj[:], ALPHA)

            with tc.tile_pool(name="stage1", bufs=1) as P1:
                # own-block fi scaled by -0.8, broadcast across partitions
                pfo = PsS.tile([16, R], F32, tag="ps_s", bufs=2)
                for fc in range(FC):
                    nc.tensor.matmul(
                        pfo[:], sw12[:, fc, :], sxTb[:, fc, :],
                        start=(fc == 0), stop=(fc == 3))
                sfob = P1.tile([16, R], BF16, name="sfob")
                nc.vector.tensor_scalar_mul(sfob[:], pfo[:], -0.8)
                fcat = P1.tile([1, NHEADS * R], BF16, name="fcat")
                nc.sync.dma_start(fcat[:], sfob[8:16, :])
                nc.gpsimd.partition_broadcast(fib08[:], fcat[:])
                prep_jc(0)
                prep_jc(1)
                prep_jc(2)
                prep_jc(3)

            # DRAM staging for the gather
            dblk2 = Pd.tile([R, 130], BF16, name="dblk2")
            dgath2 = Pd.tile([N, 130], BF16, name="dgath2",
                             addr_space="Shared")
            sfi2r = Pp.tile([1, R], BF16, name="sfi2r")

            # ================= layer-1 attention sweeps =================
            with tc.tile_pool(name="chunkL1", bufs=1) as Pc:
                def issue_em(sweep, pr):
                    em8 = Pc.tile([128, 2, 2048], BF16, tag="em8", bufs=3,
                                  name=f"em8_{sweep}_{pr}")
                    for i in range(2):
                        jc = 2 * pr + i
                        for q, hd in enumerate(
                                range(sweep * 4, sweep * 4 + 4)):
                            nc.vector.tensor_scalar(
                                em8[:, i, q * 512:(q + 1) * 512],
                                fib08[:, hd * R:(hd + 1) * R],
                                sfjT2[:, jc, hd:hd + 1],
                                sfjT[:, jc, hd:hd + 1],
                                op0=ALU.add, op1=ALU.max)
                    return em8

                paggs = {}
                saggs = {}

                def evac_stage(sweep):
                    """Stage sweep aggregates out of PSUM (frees the banks),
                    extract Z rows."""
                    for q, hd in enumerate(range(sweep * 4, sweep * 4 + 4)):
                        sagg = Pc.tile([NHID + 1, R], F32, tag=f"sagg{q}",
                                       bufs=1, name=f"sagg{hd}")
                        eng = nc.vector if q % 2 == 0 else nc.gpsimd
                        eng.tensor_copy(sagg[:], paggs[hd][:])
                        nc.gpsimd.dma_start(
                            szall[hd:hd + 1, :], sagg[NHID:NHID + 1, :])
                        saggs[hd] = sagg

                def evac_norm(sweep):
                    """Reciprocal + broadcast + normalized xcat blocks."""
                    rzf = Pc.tile([8, R], F32R, tag="rzf", bufs=2,
                                  name=f"rzf{sweep}")
                    rzs = Pc.tile([4, R], F32, tag="rzs", bufs=2)
                    nc.vector.memset(rzf[:].bitcast(F32), 0.0)
                    nc.vector.reciprocal_approx_accurate(
                        rzf[4 * sweep:4 * sweep + 4, :].bitcast(F32),
                        szall[4 * sweep:4 * sweep + 4, :], rzs[:])
                    for q, hd in enumerate(range(sweep * 4, sweep * 4 + 4)):
                        zb = PsS.tile([64, R], F32, tag="ps_s", bufs=2,
                                      name=f"zb{hd}")
                        nc.tensor.matmul(
                            zb[:], sel[:, hd * 64:(hd + 1) * 64], rzf[:],
                            start=True, stop=True)
                        eng = nc.vector if q % 2 == 0 else nc.gpsimd
                        eng.scalar_tensor_tensor(
                            sxc[:, hd, :], saggs[hd][0:NHID, :], 1.0, zb[:],
                            op0=ALU.bypass, op1=ALU.mult)

                def project_half(sweep):
                    """pH partial for this sweep's 4 heads -> dblk[sweep]."""
                    for ic in range(4):
                        pH = PsA.tile([128, 130], F32, tag="ps_a", bufs=2,
                                      name=f"pH{sweep}_{ic}")
                        for k, hd in enumerate(
                                range(sweep * 4, sweep * 4 + 4)):
                            nc.tensor.matmul(
                                pH[:], sxc[:, hd, ic * 128:(ic + 1) * 128],
                                sWoutP[:, hd, :],
                                start=(k == 0), stop=(k == 3))
                        sh2b = Pp.tile([128, 130], BF16, tag="sh2b", bufs=2)
                        nc.vector.tensor_copy(sh2b[:], pH[:])
                        nc.gpsimd.dma_start(
                            sfi2h[sweep][:, ic * 128:(ic + 1) * 128],
                            sh2b[:, 129:130])
                        nc.sync.dma_start(
                            dblk[sweep][ic * 128:(ic + 1) * 128, :],
                            sh2b[:])

                def gather_half(sweep):
                    nc.gpsimd.collective_compute(
                        "AllGather", ALU.bypass,
                        replica_groups=[list(range(NCORES))],
                        ins=[dblk[sweep][:].opt()],
                        outs=[dgath[sweep][:].opt()])

                em_cur = issue_em(0, 0)
                for sweep in range(2):
                    heads = list(range(sweep * 4, sweep * 4 + 4))
                    for pr in range(NP):
                        if pr + 1 < NP:
                            em_next = issue_em(sweep, pr + 1)
                        elif sweep == 0:
                            em_next = issue_em(1, 0)
                        P4e = Pc.tile([128, 2, 2048], BF16, tag="p4e",
                                      bufs=2)
                        P4m = Pc.tile([128, 2, 2048], BF16, tag="p4m",
                                      bufs=2)
                        nc.scalar.activation(
                            P4e[:].rearrange("p i c -> p (i c)"),
                            em_cur[:].rearrange("p i c -> p (i c)"), AF.Exp)
                        em_cur = em_next
                        if sweep == 0:
                            for dj in (4, 5):
                                if 2 * pr + dj < JC:
                                    prep_jc(2 * pr + dj)
                        for i in range(2):
                            jc = 2 * pr + i
                            for q, hd in enumerate(heads):
                                sl = slice(q * 512, (q + 1) * 512)
                                if i * 4 + q < 6:
                                    nc.vector.tensor_tensor(
                                        P4m[:, i, sl], P4e[:, i, sl],
                                        masks[:, jc, :], op=ALU.mult)
                                else:
                                    nc.gpsimd.scalar_tensor_tensor(
                                        P4m[:, i, sl], P4e[:, i, sl], 1.0,
                                        masks[:, jc, :],
                                        op0=ALU.bypass, op1=ALU.mult)
                        for i in range(2):
                            jc = 2 * pr + i
                            for q, hd in enumerate(heads):
                                if jc == 0:
                                    paggs[hd] = Pagg.tile(
                                        [NHID + 1, 512], F32,
                                        tag=f"agg{hd % 4}", bufs=1,
                                        name=f"agg_s{sweep}_{hd}")
                                nc.tensor.matmul(
                                    paggs[hd][:], shplus[:, jc, hd, :],
                                    P4m[:, i, q * 512:(q + 1) * 512],
                                    start=(jc == 0), stop=(jc == JC - 1))
                        # interleave sweep-0 epilogue into sweep 1 so the
                        # first half-gather hides under sweep-1 compute
                        if sweep == 1:
                            if pr == 0:
                                evac_stage(0)
                            elif pr == 1:
                                evac_norm(0)

                    if sweep == 1:
                        evac_stage(1)
                        evac_norm(1)
                        project_all()
                        nc.gpsimd.collective_compute(
                            "AllGather", ALU.bypass,
                            replica_groups=[list(range(NCORES))],
                            ins=[dblk2[:].opt()], outs=[dgath2[:].opt()])
                        # keep the PE hot across the exposed gather
                        warm_pe(340, "agg2", masks[:, 0, :])

            # ======================== layer 2 ========================
            with tc.tile_pool(name="stage2", bufs=1) as P2:
                # own -0.8*fi2 broadcast
                sfi2s = P2.tile([1, R], BF16, name="sfi2s")
                nc.vector.tensor_scalar_mul(sfi2s[:], sfi2r[:], -0.8)
                fib208 = P2.tile([128, R], BF16, name="fib208")
                nc.gpsimd.partition_broadcast(fib208[:], sfi2s[:])

                sh2r = P2.tile([128, JC, 130], BF16, name="sh2r")
                sfj2 = P2.tile([128, JC], F32, name="sfj2")
                sfj22 = P2.tile([128, JC], F32, name="sfj22")
                for hh in range(4):
                    hsl = slice(hh * 8, (hh + 1) * 8)
                    nc.sync.dma_start(
                        sh2r[:, hsl, :],
                        dgath2[hh * 1024:(hh + 1) * 1024, :]
                        .rearrange("(jc p) c -> p jc c", p=128))
                    nc.vector.tensor_copy(sfj2[:, hsl], sh2r[:, hsl, 128])
                    nc.vector.tensor_scalar_mul(
                        sfj22[:, hsl], sfj2[:, hsl], ALPHA)

                def issue_em2(g):
                    em8 = P2.tile([128, 8, 512], BF16, tag="em8b", bufs=2,
                                  name=f"em8b_{g}")
                    for q in range(8):
                        jc = g * 8 + q
                        nc.vector.tensor_scalar(
                            em8[:, q, :], fib208[:],
                            sfj22[:, jc:jc + 1],
                            sfj2[:, jc:jc + 1],
                            op0=ALU.add, op1=ALU.max)
                    return em8

                pagg2 = Pagg.tile([128, 512], F32, tag="agg0", bufs=1)
                pZ2 = Pagg.tile([1, 512], F32, tag="agg1", bufs=1)
                em2_cur = issue_em2(0)
                for g in range(4):
                    if g + 1 < 4:
                        em2_next = issue_em2(g + 1)
                    P4e = P2.tile([128, 8, 512], BF16, tag="p4eb", bufs=2)
                    P4m = P2.tile([128, 8, 512], BF16, tag="p4mb", bufs=2)
                    nc.scalar.activation(
                        P4e[:].rearrange("p i c -> p (i c)"),
                        em2_cur[:].rearrange("p i c -> p (i c)"), AF.Exp)
                    em2_cur = em2_next
                    for q in range(8):
                        jc = g * 8 + q
                        nc.vector.tensor_tensor(
                            P4m[:, q, :], P4e[:, q, :], masks[:, jc, :],
                            op=ALU.mult)
                    for q in range(8):
                        jc = g * 8 + q
                        nc.tensor.matmul(
                            pZ2[:], onescol[:], P4m[:, q, :],
                            start=(jc == 0), stop=(jc == JC - 1))
                        nc.tensor.matmul(
                            pagg2[:], sh2r[:, jc, 0:128], P4m[:, q, :],
                            start=(jc == 0), stop=(jc == JC - 1))

                # normalize, elu, transpose, log-softmax
                srz2 = P2.tile([1, R], F32, name="srz2")
                srz2s = P2.tile([1, R], F32, name="srz2s")
                nc.vector.reciprocal_approx_accurate(
                    srz2[:], pZ2[0:1, :], srz2s[:])
                srz2b = P2.tile([1, R], BF16, name="srz2b")
                nc.vector.tensor_copy(srz2b[:], srz2[:])
                zb2 = PsS.tile([128, R], F32, tag="ps_s", bufs=2,
                               name="zb2")
                nc.tensor.matmul(zb2[:], ones1[:], srz2b[:],
                                 start=True, stop=True)
                zb2s = P2.tile([128, R], F32, name="zb2s")
                nc.vector.tensor_copy(zb2s[:], zb2[:])
                sv = P2.tile([128, R], F32, name="sv")
                nc.vector.scalar_tensor_tensor(
                    sv[:], pagg2[:], 1.0, zb2s[:],
                    op0=ALU.bypass, op1=ALU.mult)
                smin = P2.tile([128, R], F32, name="smin")
                nc.vector.tensor_scalar_min(smin[:], sv[:], 0.0)
                sex = P2.tile([128, R], F32, name="sex")
                nc.scalar.activation(sex[:], smin[:], AF.Exp)
                srel = P2.tile([128, R], F32, name="srel")
                nc.vector.tensor_scalar_max(srel[:], sv[:], 0.0)
                sres = P2.tile([128, R], F32, name="sres")
                nc.vector.scalar_tensor_tensor(
                    sres[:], sex[:], -1.0, srel[:],
                    op0=ALU.add, op1=ALU.add)

                mx4 = P2.tile([128, 4], F32, name="mx4")
                negmx4 = P2.tile([128, 4], F32, name="negmx4")
                ssum4 = P2.tile([128, 4], F32, name="ssum4")
                sln4 = P2.tile([128, 4], F32, name="sln4")
                b24 = P2.tile([128, 4], F32, name="b24")
                st4 = P2.tile([128, 4, 128], F32, name="st4")
                sout4 = P2.tile([128, 4, 128], F32, name="sout4")
                for it in range(4):
                    ptp = PsS.tile([128, 128], F32, tag="ps_s", bufs=2,
                                   name=f"ptp{it}")
                    nc.tensor.transpose(
                        ptp[:], sres[:, it * 128:(it + 1) * 128],
                        ident[:])
                    nc.vector.tensor_copy(st4[:, it, :], ptp[:])
                nc.vector.tensor_reduce(
                    mx4[:], st4[:], axis=mybir.AxisListType.X, op=ALU.max)
                nc.vector.tensor_scalar_mul(negmx4[:], mx4[:], -1.0)
                for it in range(4):
                    sexp = P2.tile([128, 128], F32, tag="sexp", bufs=2,
                                   name=f"sexp{it}")
                    nc.scalar.activation(
                        sexp[:], st4[:, it, :], AF.Exp,
                        bias=negmx4[:, it:it + 1],
                        accum_out=ssum4[:, it:it + 1])
                nc.scalar.activation(sln4[:], ssum4[:], AF.Ln)
                nc.vector.tensor_sub(b24[:], negmx4[:], sln4[:])
                for it in range(4):
                    nc.scalar.activation(sout4[:, it, :], st4[:, it, :],
                                         AF.Identity,
                                         bias=b24[:, it:it + 1])
                    nc.sync.dma_start(
                        out_d.ap()[it * 128:(it + 1) * 128, :],
                        sout4[:, it, :])

    nc.finalize()
    return nc


def _get_nc():
    if "nc" not in _CACHE:
        _CACHE["nc"] = _build_nc()
    return _CACHE["nc"]


def kernel(**inputs):
    x = np.asarray(inputs["x"], dtype=np.float32)
    adj = np.asarray(inputs["adj"])
    W = np.asarray(inputs["W"], dtype=np.float32)
    a = np.asarray(inputs["a"], dtype=np.float32)
    W_out = np.asarray(inputs["W_out"], dtype=np.float32)
    a_out = np.asarray(inputs["a_out"], dtype=np.float32)

    bf = ml_dtypes.bfloat16
    xT = np.ascontiguousarray(x.T).astype(bf)
    Wcat = np.ascontiguousarray(
        W.transpose(1, 0, 2).reshape(NFEAT, 512))
    A12 = np.zeros((512, 16), np.float32)
    for hd in range(NHEADS):
        A12[hd * NHID:(hd + 1) * NHID, hd] = a[hd, NHID:]      # a2 -> fj
        A12[hd * NHID:(hd + 1) * NHID, 8 + hd] = a[hd, :NHID]  # a1 -> fi
    W12 = (Wcat @ A12).astype(bf)
    AO = np.stack([a_out[NCLASS:], a_out[:NCLASS]], axis=1)    # [fj, fi]
    WoutP = np.concatenate([W_out, W_out @ AO], axis=1).astype(bf)
    ident = np.eye(128, dtype=np.float32)
    sel4 = np.zeros((4, 512), np.float32)
    for q in range(4):
        sel4[q, q * 64:(q + 1) * 64] = 1.0
    sel4 = sel4.astype(bf)
    adjf = adj.astype(np.float32)

    in_maps = []
    for c in range(NCORES):
        r0, r1 = c * R, (c + 1) * R
        in_maps.append({
            "xT": xT,
            "xTblk": np.ascontiguousarray(x[r0:r1].T).astype(bf),
            "Wcat": Wcat.astype(bf),
            "W12": W12,
            "WoutP": WoutP,
            "adjT": np.ascontiguousarray(adjf[r0:r1].T).astype(bf),
            "sel4": sel4,
            "ident": ident,
        })

    nc = _get_nc()
    trace = bool(os.environ.get("KERNEL_TRACE"))
    res = bass_utils.run_bass_kernel_spmd(
        nc, in_maps, list(range(NCORES)), trace=trace)
    kernel.last_results = res
    out = np.concatenate(
        [res.results[c]["out"] for c in range(NCORES)], axis=0)
    return np.ascontiguousarray(out, dtype=np.float32)


# revision 51
# speedup vs baseline: 1.6021x; 1.0066x over previous
"""GAT (2-layer multi-head graph attention) on 8 Trainium2 NeuronCores.

Sharding: nodes (rows of adj / attention) are sharded across the 8 cores;
each core computes h = x@W replicated, its 512-row block of
e/softmax/aggregation for both GAT layers.  The layer-1 -> layer-2
projections (h2|fj2|fi2 packed as 130 cols) are AllGathered in TWO halves:
the heads-0..3 partial right after sweep 0 (hidden under sweep 1), the
heads-4..7 partial at the end; layer 2 sums the gathered partials.

Layout: attention probabilities are computed TRANSPOSED (eT[j, i]) so the
softmax-normalizer and aggregation both run on the tensor engine via an
hplus = [h | 1] stationary operand (last row of the accumulator is Z).

Key factorization: softmax over j is invariant to any per-i shift, so we
compute e'[j,i] = leakyrelu(fi[i]+fj[j]) - fi[i] = max(fj, 0.2*fj - 0.8*fi).
With fib08 = -0.8*fi broadcast tiles precomputed once per head, the whole
e-map is ONE tensor_scalar (DVE 4x mode) per tile plus exp plus mask:
  em = (fib08 + 0.2*fj) max fj  (tensor_scalar, two ptr scalars, DVE 4x)
  P  = exp(em)                  (scalar engine; one [128,4096] op per 2 jc)
  Pm = P * adj                  (tensor_tensor 2x, split 6:2 DVE/Pool)
The 0/1 adjacency multiply replaces the -1e9 mask (exact zeros).
Weight-only transforms (Wcat, w12 = Wcat @ A12, w2 appended to Wout) are
folded on the host.  Z-reciprocal broadcasts are built with one-hot outer
products on the tensor engine.  Idle-window dummy matmuls keep the PE
p-state warm across the exposed collective.
"""
import os
import sys

for _p in ("/opt/trn_rl_repo", "/root/.axon_site/_ro/trn_rl_repo"):
    if os.path.isdir(_p) and _p not in sys.path:
        sys.path.insert(0, _p)

import numpy as np
import ml_dtypes

import concourse.bacc as bacc
import concourse.mybir as mybir
import concourse.tile as tile
from concourse import bass_utils

F32 = mybir.dt.float32
F32R = mybir.dt.float32r
BF16 = mybir.dt.bfloat16
AF = mybir.ActivationFunctionType
ALU = mybir.AluOpType

N, NFEAT, NHID, NCLASS, NHEADS = 4096, 512, 64, 128, 8
NCORES = 8
R = N // NCORES          # 512 rows per core
FC = NFEAT // 128        # 4 feature chunks
JC = N // 128            # 32 j-chunks
NP = JC // 2             # 16 jc-pairs
NP_BUILD = int(os.environ.get('NP_BUILD', '0'))
ALPHA = 0.2

_CACHE = {}
SKIP_WARM = bool(os.environ.get('SKIP_WARM'))
SKIP_L2 = bool(os.environ.get('SKIP_L2'))
SKIP_EVAC_ACT = bool(os.environ.get('SKIP_EVAC_ACT'))


def _build_nc():
    nc = bacc.Bacc("TRN2", target_bir_lowering=False, debug=False,
                   num_devices=NCORES)

    xT_d = nc.dram_tensor("xT", [NFEAT, N], BF16, kind="ExternalInput")
    xTb_d = nc.dram_tensor("xTblk", [NFEAT, R], BF16, kind="ExternalInput")
    Wcat_d = nc.dram_tensor("Wcat", [NFEAT, 512], BF16, kind="ExternalInput")
    W12_d = nc.dram_tensor("W12", [NFEAT, 16], BF16, kind="ExternalInput")
    WoutP_d = nc.dram_tensor("WoutP", [512, 130], BF16, kind="ExternalInput")
    adj_d = nc.dram_tensor("adjT", [N, R], BF16, kind="ExternalInput")
    sel_d = nc.dram_tensor("sel4", [4, 512], BF16, kind="ExternalInput")
    id_d = nc.dram_tensor("ident", [128, 128], F32, kind="ExternalInput")
    out_d = nc.dram_tensor("out", [R, NCLASS], F32, kind="ExternalOutput")

    with tile.TileContext(nc, num_cores=NCORES) as tc:
        with (
            tc.tile_pool(name="persist", bufs=1) as Pp,
            tc.tile_pool(name="dram", bufs=1, space="DRAM") as Pd,
            tc.tile_pool(name="psA", bufs=2, space="PSUM") as PsA,
            tc.tile_pool(name="psS", bufs=2, space="PSUM") as PsS,
            tc.tile_pool(name="pagg", bufs=1, space="PSUM") as Pagg,
        ):
            # ---- persistent constants ----
            onescol = Pp.tile([128, 1], BF16, name="onescol")
            nc.vector.memset(onescol[:], 1.0)
            ones1 = Pp.tile([1, 128], BF16, name="ones1")
            nc.vector.memset(ones1[:], 1.0)
            sel = Pp.tile([4, 512], BF16, name="sel")
            nc.sync.dma_start(sel[:], sel_d.ap())

            # ---- DMAs ordered so the L1 pipeline can start ASAP ----
            sWcatF = Pp.tile([128, FC, 512], BF16, name="sWcatF")
            nc.sync.dma_start(
                sWcatF[:],
                Wcat_d.ap().rearrange("(fc p) o -> p fc o", p=128))
            sw12 = Pp.tile([128, FC, 16], BF16, name="sw12")
            nc.sync.dma_start(
                sw12[:], W12_d.ap().rearrange("(fc p) o -> p fc o", p=128))
            sxTb = Pp.tile([128, FC, R], BF16, name="sxTb")
            nc.sync.dma_start(
                sxTb[:], xTb_d.ap().rearrange("(fc p) r -> p fc r", p=128))
            ident = Pp.tile([128, 128], F32R, name="ident")
            nc.sync.dma_start(ident[:], id_d.ap())

            # x and adjacency stream in j-order, interleaved
            sx = Pp.tile([128, FC, N], BF16, name="sx")
            masks = Pp.tile([128, JC, R], BF16, name="masks")
            for g in range(4):
                nc.sync.dma_start(
                    sx[:, :, g * 1024:(g + 1) * 1024],
                    xT_d.ap()[:, g * 1024:(g + 1) * 1024]
                    .rearrange("(fc p) j -> p fc j", p=128))
                nc.sync.dma_start(
                    masks[:, g * 8:(g + 1) * 8, :],
                    adj_d.ap()[g * 1024:(g + 1) * 1024, :]
                    .rearrange("(jc p) r -> p jc r", p=128))
            sWoutP = Pp.tile([64, NHEADS, 130], BF16, name="sWoutP")
            nc.sync.dma_start(
                sWoutP[:], WoutP_d.ap().rearrange("(hd p) c -> p hd c", p=64))

            sfjT = Pp.tile([128, JC, 8], F32, name="sfjT")
            sfjT2 = Pp.tile([128, JC, 8], F32, name="sfjT2")  # 0.2 * fj
            fib08 = Pp.tile([128, NHEADS * R], BF16, name="fib08")  # -0.8*fi
            sxc = Pp.tile([64, NHEADS, R], BF16, name="sxc")
            szall = Pp.tile([8, R], F32, name="szall")

            shplus = Pp.tile([128, JC, NHEADS, NHID + 1], BF16, name="shplus")
            nc.vector.memset(shplus[:, :, :, NHID], 1.0)

            def warm_pe(n, tag, shape_src):
                """Dummy matmuls that keep the PE p-state ramped while it
                would otherwise idle (in-order PE queue = placement)."""
                w = Pagg.tile([1, 512], F32, tag=tag, bufs=1, name=f"w{tag}")
                for _ in range(n):
                    nc.tensor.matmul(w[:], onescol[:], shape_src,
                                     start=True, stop=True)

            # ================= prep: h tiles / fj / fi =================
            def prep_jc(jc):
                xa = [sx[:, fc, jc * 128:(jc + 1) * 128] for fc in range(FC)]
                pA = PsA.tile([128, 512], F32, tag="ps_a", bufs=2,
                              name=f"pA{jc}")
                for fc in range(FC):
                    nc.tensor.matmul(
                        pA[:], xa[fc], sWcatF[:, fc, :],
                        start=(fc == 0), stop=(fc == 3))
                nc.gpsimd.tensor_copy(
                    shplus[:, jc, :, 0:NHID],
                    pA[:].rearrange("p (hd o) -> p hd o", o=NHID))
                pfj = PsS.tile([128, 8], F32, tag="ps_s", bufs=2,
                               name=f"pfj{jc}")
                for fc in range(FC):
                    nc.tensor.matmul(
                        pfj[:], xa[fc], sw12[:, fc, 0:8],
                        start=(fc == 0), stop=(fc == 3))
                nc.gpsimd.tensor_copy(sfjT[:, jc, :], pfj[:])
                nc.gpsimd.tensor_scalar_mul(sfjT2[:, jc, :], pfj[:], ALPHA)

            with tc.tile_pool(name="stage1", bufs=1) as P1:
                prep_jc(0)
                prep_jc(1)
                # own-block fi scaled by -0.8, broadcast across partitions
                pfo = PsS.tile([16, R], F32, tag="ps_s", bufs=2)
                for fc in range(FC):
                    nc.tensor.matmul(
                        pfo[:], sw12[:, fc, :], sxTb[:, fc, :],
                        start=(fc == 0), stop=(fc == 3))
                sfob = P1.tile([16, R], BF16, name="sfob")
                nc.vector.tensor_scalar_mul(sfob[:], pfo[:], -0.8)
                fcat = P1.tile([1, NHEADS * R], BF16, name="fcat")
                nc.sync.dma_start(fcat[:], sfob[8:16, :])
                nc.gpsimd.partition_broadcast(fib08[:], fcat[:])
                prep_jc(2)
                prep_jc(3)

            # DRAM staging for the gather
            dblk2 = Pd.tile([R, 130], BF16, name="dblk2")
            dgath2 = Pd.tile([N, 130], BF16, name="dgath2",
                             addr_space="Shared")
            sfi2r = Pp.tile([1, R], BF16, name="sfi2r")

            # ================= layer-1 attention sweeps =================
            with tc.tile_pool(name="chunkL1", bufs=1) as Pc:
                def issue_em(sweep, pr):
                    em8 = Pc.tile([128, 2, 2048], BF16, tag="em8", bufs=3,
                                  name=f"em8_{sweep}_{pr}")
                    for i in range(2):
                        jc = 2 * pr + i
                        for q, hd in enumerate(
                                range(sweep * 4, sweep * 4 + 4)):
                            nc.vector.tensor_scalar(
                                em8[:, i, q * 512:(q + 1) * 512],
                                fib08[:, hd * R:(hd + 1) * R],
                                sfjT2[:, jc, hd:hd + 1],
                                sfjT[:, jc, hd:hd + 1],
                                op0=ALU.add, op1=ALU.max)
                    return em8

                paggs = {}
                saggs = {}

                def evac_stage(sweep):
                    """Stage sweep aggregates out of PSUM (frees the banks),
                    extract Z rows."""
                    for q, hd in enumerate(range(sweep * 4, sweep * 4 + 4)):
                        sagg = Pc.tile([NHID + 1, R], F32, tag=f"sagg{q}",
                                       bufs=1, name=f"sagg{hd}")
                        eng = nc.vector if q % 2 == 0 else nc.gpsimd
                        eng.tensor_copy(sagg[:], paggs[hd][:])
                        nc.gpsimd.dma_start(
                            szall[hd:hd + 1, :], sagg[NHID:NHID + 1, :])
                        saggs[hd] = sagg

                def evac_norm(sweep):
                    """Reciprocal + broadcast + normalized xcat blocks."""
                    rzf = Pc.tile([8, R], F32R, tag="rzf", bufs=2,
                                  name=f"rzf{sweep}")
                    rzs = Pc.tile([4, R], F32, tag="rzs", bufs=2)
                    nc.vector.memset(rzf[:].bitcast(F32), 0.0)
                    nc.vector.reciprocal_approx_accurate(
                        rzf[4 * sweep:4 * sweep + 4, :].bitcast(F32),
                        szall[4 * sweep:4 * sweep + 4, :], rzs[:])
                    for q, hd in enumerate(range(sweep * 4, sweep * 4 + 4)):
                        zb = PsS.tile([64, R], F32, tag="ps_s", bufs=2,
                                      name=f"zb{hd}")
                        nc.tensor.matmul(
                            zb[:], sel[:, hd * 64:(hd + 1) * 64], rzf[:],
                            start=True, stop=True)
                        eng = nc.vector if q % 2 == 0 else nc.gpsimd
                        eng.scalar_tensor_tensor(
                            sxc[:, hd, :], saggs[hd][0:NHID, :], 1.0, zb[:],
                            op0=ALU.bypass, op1=ALU.mult)

                def project_half(sweep):
                    """pH partial for this sweep's 4 heads -> dblk[sweep]."""
                    for ic in range(4):
                        pH = PsA.tile([128, 130], F32, tag="ps_a", bufs=2,
                                      name=f"pH{sweep}_{ic}")
                        for k, hd in enumerate(
                                range(sweep * 4, sweep * 4 + 4)):
                            nc.tensor.matmul(
                                pH[:], sxc[:, hd, ic * 128:(ic + 1) * 128],
                                sWoutP[:, hd, :],
                                start=(k == 0), stop=(k == 3))
                        sh2b = Pp.tile([128, 130], BF16, tag="sh2b", bufs=2)
                        nc.vector.tensor_copy(sh2b[:], pH[:])
                        nc.gpsimd.dma_start(
                            sfi2h[sweep][:, ic * 128:(ic + 1) * 128],
                            sh2b[:, 129:130])
                        nc.sync.dma_start(
                            dblk[sweep][ic * 128:(ic + 1) * 128, :],
                            sh2b[:])

                def gather_half(sweep):
                    nc.gpsimd.collective_compute(
                        "AllGather", ALU.bypass,
                        replica_groups=[list(range(NCORES))],
                        ins=[dblk[sweep][:].opt()],
                        outs=[dgath[sweep][:].opt()])

                em_cur = issue_em(0, 0)
                for sweep in range(2):
                    heads = list(range(sweep * 4, sweep * 4 + 4))
                    for pr in range(NP):
                        if pr + 1 < NP:
                            em_next = issue_em(sweep, pr + 1)
                        elif sweep == 0:
                            em_next = issue_em(1, 0)
                        P4e = Pc.tile([128, 2, 2048], BF16, tag="p4e",
                                      bufs=2)
                        P4m = Pc.tile([128, 2, 2048], BF16, tag="p4m",
                                      bufs=2)
                        nc.scalar.activation(
                            P4e[:].rearrange("p i c -> p (i c)"),
                            em_cur[:].rearrange("p i c -> p (i c)"), AF.Exp)
                        em_cur = em_next
                        if sweep == 0:
                            for dj in (4, 5):
                                if 2 * pr + dj < JC:
                                    prep_jc(2 * pr + dj)
                        for i in range(2):
                            jc = 2 * pr + i
                            for q, hd in enumerate(heads):
                                sl = slice(q * 512, (q + 1) * 512)
                                if i * 4 + q < 6:
                                    nc.vector.tensor_tensor(
                                        P4m[:, i, sl], P4e[:, i, sl],
                                        masks[:, jc, :], op=ALU.mult)
                                else:
                                    nc.gpsimd.scalar_tensor_tensor(
                                        P4m[:, i, sl], P4e[:, i, sl], 1.0,
                                        masks[:, jc, :],
                                        op0=ALU.bypass, op1=ALU.mult)
                        for i in range(2):
                            jc = 2 * pr + i
                            for q, hd in enumerate(heads):
                                if jc == 0:
                                    paggs[hd] = Pagg.tile(
                                        [NHID + 1, 512], F32,
                                        tag=f"agg{hd % 4}", bufs=1,
                                        name=f"agg_s{sweep}_{hd}")
                                nc.tensor.matmul(
                                    paggs[hd][:], shplus[:, jc, hd, :],
                                    P4m[:, i, q * 512:(q + 1) * 512],
                                    start=(jc == 0), stop=(jc == JC - 1))
                        # interleave sweep-0 epilogue into sweep 1 so the
                        # first half-gather hides under sweep-1 compute
                        if sweep == 1:
                            if pr == 0:
                                evac_stage(0)
                            elif pr == 1:
                                evac_norm(0)

                    if sweep == 1:
                        evac_stage(1)
                        evac_norm(1)
                        project_all()
                        nc.gpsimd.collective_compute(
                            "AllGather", ALU.bypass,
                            replica_groups=[list(range(NCORES))],
                            ins=[dblk2[:].opt()], outs=[dgath2[:].opt()])
                        # keep the PE hot across the exposed gather
                        warm_pe(340, "agg2", masks[:, 0, :])

            # ======================== layer 2 ========================
            with tc.tile_pool(name="stage2", bufs=1) as P2:
                # own -0.8*fi2 broadcast
                sfi2s = P2.tile([1, R], BF16, name="sfi2s")
                nc.vector.tensor_scalar_mul(sfi2s[:], sfi2r[:], -0.8)
                fib208 = P2.tile([128, R], BF16, name="fib208")
                nc.gpsimd.partition_broadcast(fib208[:], sfi2s[:])

                sh2r = P2.tile([128, JC, 130], BF16, name="sh2r")
                sfj2 = P2.tile([128, JC], F32, name="sfj2")
                sfj22 = P2.tile([128, JC], F32, name="sfj22")
                for hh in range(4):
                    hsl = slice(hh * 8, (hh + 1) * 8)
                    nc.sync.dma_start(
                        sh2r[:, hsl, :],
                        dgath2[hh * 1024:(hh + 1) * 1024, :]
                        .rearrange("(jc p) c -> p jc c", p=128))
                    nc.vector.tensor_copy(sfj2[:, hsl], sh2r[:, hsl, 128])
                    nc.vector.tensor_scalar_mul(
                        sfj22[:, hsl], sfj2[:, hsl], ALPHA)

                def issue_em2(g):
                    em8 = P2.tile([128, 8, 512], BF16, tag="em8b", bufs=2,
                                  name=f"em8b_{g}")
                    for q in range(8):
                        jc = g * 8 + q
                        nc.vector.tensor_scalar(
                            em8[:, q, :], fib208[:],
                            sfj22[:, jc:jc + 1],
                            sfj2[:, jc:jc + 1],
                            op0=ALU.add, op1=ALU.max)
                    return em8

                pagg2 = Pagg.tile([128, 512], F32, tag="agg0", bufs=1)
                pZ2 = Pagg.tile([1, 512], F32, tag="agg1", bufs=1)
                em2_cur = issue_em2(0)
                for g in range(4):
                    if g + 1 < 4:
                        em2_next = issue_em2(g + 1)
                    P4e = P2.tile([128, 8, 512], BF16, tag="p4eb", bufs=2)
                    P4m = P2.tile([128, 8, 512], BF16, tag="p4mb", bufs=2)
                    nc.scalar.activation(
                        P4e[:].rearrange("p i c -> p (i c)"),
                        em2_cur[:].rearrange("p i c -> p (i c)"), AF.Exp)
                    em2_cur = em2_next
                    for q in range(8):
                        jc = g * 8 + q
                        nc.vector.tensor_tensor(
                            P4m[:, q, :], P4e[:, q, :], masks[:, jc, :],
                            op=ALU.mult)
                    for q in range(8):
                        jc = g * 8 + q
                        nc.tensor.matmul(
                            pZ2[:], onescol[:], P4m[:, q, :],
                            start=(jc == 0), stop=(jc == JC - 1))
                        nc.tensor.matmul(
                            pagg2[:], sh2r[:, jc, 0:128], P4m[:, q, :],
                            start=(jc == 0), stop=(jc == JC - 1))

                # normalize, elu, transpose, log-softmax
                srz2 = P2.tile([1, R], F32, name="srz2")
                srz2s = P2.tile([1, R], F32, name="srz2s")
                nc.vector.reciprocal_approx_accurate(
                    srz2[:], pZ2[0:1, :], srz2s[:])
                srz2b = P2.tile([1, R], BF16, name="srz2b")
                nc.vector.tensor_copy(srz2b[:], srz2[:])
                zb2 = PsS.tile([128, R], F32, tag="ps_s", bufs=2,
                               name="zb2")
                nc.tensor.matmul(zb2[:], ones1[:], srz2b[:],
                                 start=True, stop=True)
                zb2s = P2.tile([128, R], F32, name="zb2s")
                sv = P2.tile([128, R], F32, name="sv")
                smin = P2.tile([128, R], F32, name="smin")
                sex = P2.tile([128, R], F32, name="sex")
                srel = P2.tile([128, R], F32, name="srel")
                sres = P2.tile([128, R], F32, name="sres")
                for hh in range(2):
                    hs = slice(hh * 256, (hh + 1) * 256)
                    nc.vector.tensor_copy(zb2s[:, hs], zb2[:, hs])
                    nc.vector.scalar_tensor_tensor(
                        sv[:, hs], pagg2[:, hs], 1.0, zb2s[:, hs],
                        op0=ALU.bypass, op1=ALU.mult)
                    nc.vector.tensor_scalar_min(
                        smin[:, hs], sv[:, hs], 0.0)
                    nc.scalar.activation(sex[:, hs], smin[:, hs], AF.Exp)
                    nc.vector.tensor_scalar_max(
                        srel[:, hs], sv[:, hs], 0.0)
                    nc.vector.scalar_tensor_tensor(
                        sres[:, hs], sex[:, hs], -1.0, srel[:, hs],
                        op0=ALU.add, op1=ALU.add)

                mx4 = P2.tile([128, 4], F32, name="mx4")
                negmx4 = P2.tile([128, 4], F32, name="negmx4")
                ssum4 = P2.tile([128, 4], F32, name="ssum4")
                sln4 = P2.tile([128, 4], F32, name="sln4")
                b24 = P2.tile([128, 4], F32, name="b24")
                st4 = P2.tile([128, 4, 128], F32, name="st4")
                sout4 = P2.tile([128, 4, 128], F32, name="sout4")
                for it in range(4):
                    ptp = PsS.tile([128, 128], F32, tag="ps_s", bufs=2,
                                   name=f"ptp{it}")
                    nc.tensor.transpose(
                        ptp[:], sres[:, it * 128:(it + 1) * 128],
                        ident[:])
                    nc.vector.tensor_copy(st4[:, it, :], ptp[:])
                nc.vector.tensor_reduce(
                    mx4[:], st4[:], axis=mybir.AxisListType.X, op=ALU.max)
                nc.vector.tensor_scalar_mul(negmx4[:], mx4[:], -1.0)
                for it in range(4):
                    sexp = P2.tile([128, 128], F32, tag="sexp", bufs=2,
                                   name=f"sexp{it}")
                    nc.scalar.activation(
                        sexp[:], st4[:, it, :], AF.Exp,
                        bias=negmx4[:, it:it + 1],
                        accum_out=ssum4[:, it:it + 1])
                nc.scalar.activation(sln4[:], ssum4[:], AF.Ln)
                nc.vector.tensor_sub(b24[:], negmx4[:], sln4[:])
                for it in range(4):
                    nc.scalar.activation(sout4[:, it, :], st4[:, it, :],
                                         AF.Identity,
                                         bias=b24[:, it:it + 1])
                    nc.sync.dma_start(
                        out_d.ap()[it * 128:(it + 1) * 128, :],
                        sout4[:, it, :])

    nc.finalize()
    return nc


def _get_nc():
    if "nc" not in _CACHE:
        _CACHE["nc"] = _build_nc()
    return _CACHE["nc"]


def kernel(**inputs):
    x = np.asarray(inputs["x"], dtype=np.float32)
    adj = np.asarray(inputs["adj"])
    W = np.asarray(inputs["W"], dtype=np.float32)
    a = np.asarray(inputs["a"], dtype=np.float32)
    W_out = np.asarray(inputs["W_out"], dtype=np.float32)
    a_out = np.asarray(inputs["a_out"], dtype=np.float32)

    bf = ml_dtypes.bfloat16
    xT = np.ascontiguousarray(x.T).astype(bf)
    Wcat = np.ascontiguousarray(
        W.transpose(1, 0, 2).reshape(NFEAT, 512))
    A12 = np.zeros((512, 16), np.float32)
    for hd in range(NHEADS):
        A12[hd * NHID:(hd + 1) * NHID, hd] = a[hd, NHID:]      # a2 -> fj
        A12[hd * NHID:(hd + 1) * NHID, 8 + hd] = a[hd, :NHID]  # a1 -> fi
    W12 = (Wcat @ A12).astype(bf)
    AO = np.stack([a_out[NCLASS:], a_out[:NCLASS]], axis=1)    # [fj, fi]
    WoutP = np.concatenate([W_out, W_out @ AO], axis=1).astype(bf)
    ident = np.eye(128, dtype=np.float32)
    sel4 = np.zeros((4, 512), np.float32)
    for q in range(4):
        sel4[q, q * 64:(q + 1) * 64] = 1.0
    sel4 = sel4.astype(bf)
    adjf = adj.astype(np.float32)

    in_maps = []
    for c in range(NCORES):
        r0, r1 = c * R, (c + 1) * R
        in_maps.append({
            "xT": xT,
            "xTblk": np.ascontiguousarray(x[r0:r1].T).astype(bf),
            "Wcat": Wcat.astype(bf),
            "W12": W12,
            "WoutP": WoutP,
            "adjT": np.ascontiguousarray(adjf[r0:r1].T).astype(bf),
            "sel4": sel4,
            "ident": ident,
        })

    nc = _get_nc()
    trace = bool(os.environ.get("KERNEL_TRACE"))
    res = bass_utils.run_bass_kernel_spmd(
        nc, in_maps, list(range(NCORES)), trace=trace)
    kernel.last_results = res
    out = np.concatenate(
        [res.results[c]["out"] for c in range(NCORES)], axis=0)
    return np.ascontiguousarray(out, dtype=np.float32)


# revision 55
# speedup vs baseline: 1.6051x; 1.0019x over previous
"""GAT (2-layer multi-head graph attention) on 8 Trainium2 NeuronCores.

Sharding: nodes (rows of adj / attention) are sharded across the 8 cores;
each core computes h = x@W replicated, its 512-row block of
e/softmax/aggregation for both GAT layers.  The layer-1 -> layer-2
projections (h2|fj2|fi2 packed as 130 cols) are AllGathered in TWO halves:
the heads-0..3 partial right after sweep 0 (hidden under sweep 1), the
heads-4..7 partial at the end; layer 2 sums the gathered partials.

Layout: attention probabilities are computed TRANSPOSED (eT[j, i]) so the
softmax-normalizer and aggregation both run on the tensor engine via an
hplus = [h | 1] stationary operand (last row of the accumulator is Z).

Key factorization: softmax over j is invariant to any per-i shift, so we
compute e'[j,i] = leakyrelu(fi[i]+fj[j]) - fi[i] = max(fj, 0.2*fj - 0.8*fi).
With fib08 = -0.8*fi broadcast tiles precomputed once per head, the whole
e-map is ONE tensor_scalar (DVE 4x mode) per tile plus exp plus mask:
  em = (fib08 + 0.2*fj) max fj  (tensor_scalar, two ptr scalars, DVE 4x)
  P  = exp(em)                  (scalar engine; one [128,4096] op per 2 jc)
  Pm = P * adj                  (tensor_tensor 2x, split 6:2 DVE/Pool)
The 0/1 adjacency multiply replaces the -1e9 mask (exact zeros).
Weight-only transforms (Wcat, w12 = Wcat @ A12, w2 appended to Wout) are
folded on the host.  Z-reciprocal broadcasts are built with one-hot outer
products on the tensor engine.  Idle-window dummy matmuls keep the PE
p-state warm across the exposed collective.
"""
import os
import sys

for _p in ("/opt/trn_rl_repo", "/root/.axon_site/_ro/trn_rl_repo"):
    if os.path.isdir(_p) and _p not in sys.path:
        sys.path.insert(0, _p)

import numpy as np
import ml_dtypes

import concourse.bacc as bacc
import concourse.mybir as mybir
import concourse.tile as tile
from concourse import bass_utils

F32 = mybir.dt.float32
F32R = mybir.dt.float32r
BF16 = mybir.dt.bfloat16
AF = mybir.ActivationFunctionType
ALU = mybir.AluOpType

N, NFEAT, NHID, NCLASS, NHEADS = 4096, 512, 64, 128, 8
NCORES = 8
R = N // NCORES          # 512 rows per core
FC = NFEAT // 128        # 4 feature chunks
JC = N // 128            # 32 j-chunks
NP = JC // 2             # 16 jc-pairs
NP_BUILD = int(os.environ.get('NP_BUILD', '0'))
ALPHA = 0.2

_CACHE = {}
SKIP_WARM = bool(os.environ.get('SKIP_WARM'))
SKIP_L2 = bool(os.environ.get('SKIP_L2'))
SKIP_EVAC_ACT = bool(os.environ.get('SKIP_EVAC_ACT'))


def _build_nc():
    nc = bacc.Bacc("TRN2", target_bir_lowering=False, debug=False,
                   num_devices=NCORES)

    xT_d = nc.dram_tensor("xT", [NFEAT, N], BF16, kind="ExternalInput")
    xTb_d = nc.dram_tensor("xTblk", [NFEAT, R], BF16, kind="ExternalInput")
    Wcat_d = nc.dram_tensor("Wcat", [NFEAT, 512], BF16, kind="ExternalInput")
    W12_d = nc.dram_tensor("W12", [NFEAT, 16], BF16, kind="ExternalInput")
    WoutP_d = nc.dram_tensor("WoutP", [512, 130], BF16, kind="ExternalInput")
    adj_d = nc.dram_tensor("adjT", [N, R], BF16, kind="ExternalInput")
    sel_d = nc.dram_tensor("sel4", [4, 512], BF16, kind="ExternalInput")
    id_d = nc.dram_tensor("ident", [128, 128], F32, kind="ExternalInput")
    out_d = nc.dram_tensor("out", [R, NCLASS], F32, kind="ExternalOutput")

    with tile.TileContext(nc, num_cores=NCORES) as tc:
        with (
            tc.tile_pool(name="persist", bufs=1) as Pp,
            tc.tile_pool(name="dram", bufs=1, space="DRAM") as Pd,
            tc.tile_pool(name="psA", bufs=2, space="PSUM") as PsA,
            tc.tile_pool(name="psS", bufs=2, space="PSUM") as PsS,
            tc.tile_pool(name="pagg", bufs=1, space="PSUM") as Pagg,
        ):
            # ---- persistent constants ----
            onescol = Pp.tile([128, 1], BF16, name="onescol")
            nc.vector.memset(onescol[:], 1.0)
            ones1 = Pp.tile([1, 128], BF16, name="ones1")
            nc.vector.memset(ones1[:], 1.0)
            sel = Pp.tile([4, 512], BF16, name="sel")
            nc.sync.dma_start(sel[:], sel_d.ap())

            # ---- DMAs ordered so the L1 pipeline can start ASAP ----
            sWcatF = Pp.tile([128, FC, 512], BF16, name="sWcatF")
            nc.sync.dma_start(
                sWcatF[:],
                Wcat_d.ap().rearrange("(fc p) o -> p fc o", p=128))
            sw12 = Pp.tile([128, FC, 16], BF16, name="sw12")
            nc.sync.dma_start(
                sw12[:], W12_d.ap().rearrange("(fc p) o -> p fc o", p=128))
            sxTb = Pp.tile([128, FC, R], BF16, name="sxTb")
            nc.sync.dma_start(
                sxTb[:], xTb_d.ap().rearrange("(fc p) r -> p fc r", p=128))
            ident = Pp.tile([128, 128], F32R, name="ident")
            nc.sync.dma_start(ident[:], id_d.ap())

            # x and adjacency stream in j-order, interleaved
            sx = Pp.tile([128, FC, N], BF16, name="sx")
            masks = Pp.tile([128, JC, R], BF16, name="masks")
            for g in range(4):
                nc.sync.dma_start(
                    sx[:, :, g * 1024:(g + 1) * 1024],
                    xT_d.ap()[:, g * 1024:(g + 1) * 1024]
                    .rearrange("(fc p) j -> p fc j", p=128))
                nc.sync.dma_start(
                    masks[:, g * 8:(g + 1) * 8, :],
                    adj_d.ap()[g * 1024:(g + 1) * 1024, :]
                    .rearrange("(jc p) r -> p jc r", p=128))
            sWoutP = Pp.tile([64, NHEADS, 130], BF16, name="sWoutP")
            nc.sync.dma_start(
                sWoutP[:], WoutP_d.ap().rearrange("(hd p) c -> p hd c", p=64))

            sfjT = Pp.tile([128, JC, 8], F32, name="sfjT")
            sfjT2 = Pp.tile([128, JC, 8], F32, name="sfjT2")  # 0.2 * fj
            fib08 = Pp.tile([128, NHEADS * R], BF16, name="fib08")  # -0.8*fi
            sxc = Pp.tile([64, NHEADS, R], BF16, name="sxc")
            szall = Pp.tile([8, R], F32, name="szall")

            shplus = Pp.tile([128, JC, NHEADS, NHID + 1], BF16, name="shplus")
            nc.vector.memset(shplus[:, :, :, NHID], 1.0)

            def warm_pe(n, tag, shape_src):
                """Dummy matmuls that keep the PE p-state ramped while it
                would otherwise idle (in-order PE queue = placement)."""
                w = Pagg.tile([1, 512], F32, tag=tag, bufs=1, name=f"w{tag}")
                for _ in range(n):
                    nc.tensor.matmul(w[:], onescol[:], shape_src,
                                     start=True, stop=True)

            # ================= prep: h tiles / fj / fi =================
            def prep_jc(jc):
                xa = [sx[:, fc, jc * 128:(jc + 1) * 128] for fc in range(FC)]
                pA = PsA.tile([128, 512], F32, tag="ps_a", bufs=2,
                              name=f"pA{jc}")
                for fc in range(FC):
                    nc.tensor.matmul(
                        pA[:], xa[fc], sWcatF[:, fc, :],
                        start=(fc == 0), stop=(fc == 3))
                nc.gpsimd.tensor_copy(
                    shplus[:, jc, :, 0:NHID],
                    pA[:].rearrange("p (hd o) -> p hd o", o=NHID))
                pfj = PsS.tile([128, 8], F32, tag="ps_s", bufs=2,
                               name=f"pfj{jc}")
                for fc in range(FC):
                    nc.tensor.matmul(
                        pfj[:], xa[fc], sw12[:, fc, 0:8],
                        start=(fc == 0), stop=(fc == 3))
                nc.gpsimd.tensor_copy(sfjT[:, jc, :], pfj[:])
                nc.gpsimd.tensor_scalar_mul(sfjT2[:, jc, :], pfj[:], ALPHA)

            with tc.tile_pool(name="stage1", bufs=1) as P1:
                # own-block fi scaled by -0.8, broadcast across partitions
                pfo = PsS.tile([16, R], F32, tag="ps_s", bufs=2)
                for fc in range(FC):
                    nc.tensor.matmul(
                        pfo[:], sw12[:, fc, :], sxTb[:, fc, :],
                        start=(fc == 0), stop=(fc == 3))
                sfob = P1.tile([16, R], BF16, name="sfob")
                nc.vector.tensor_scalar_mul(sfob[:], pfo[:], -0.8)
                fcat = P1.tile([1, NHEADS * R], BF16, name="fcat")
                nc.sync.dma_start(fcat[:], sfob[8:16, :])
                nc.gpsimd.partition_broadcast(fib08[:], fcat[:])
                prep_jc(0)
                prep_jc(1)
                prep_jc(2)
                prep_jc(3)

            # DRAM staging for the gather
            dblk2 = Pd.tile([R, 130], BF16, name="dblk2")
            dgath2 = Pd.tile([N, 130], BF16, name="dgath2",
                             addr_space="Shared")
            sfi2r = Pp.tile([1, R], BF16, name="sfi2r")

            # ================= layer-1 attention sweeps =================
            with tc.tile_pool(name="chunkL1", bufs=1) as Pc:
                def issue_em(sweep, pr):
                    em8 = Pc.tile([128, 2, 2048], BF16, tag="em8", bufs=3,
                                  name=f"em8_{sweep}_{pr}")
                    for i in range(2):
                        jc = 2 * pr + i
                        for q, hd in enumerate(
                                range(sweep * 4, sweep * 4 + 4)):
                            nc.vector.tensor_scalar(
                                em8[:, i, q * 512:(q + 1) * 512],
                                fib08[:, hd * R:(hd + 1) * R],
                                sfjT2[:, jc, hd:hd + 1],
                                sfjT[:, jc, hd:hd + 1],
                                op0=ALU.add, op1=ALU.max)
                    return em8

                paggs = {}
                saggs = {}

                def evac_stage(sweep):
                    """Stage sweep aggregates out of PSUM (frees the banks),
                    extract Z rows."""
                    for q, hd in enumerate(range(sweep * 4, sweep * 4 + 4)):
                        sagg = Pc.tile([NHID + 1, R], F32, tag=f"sagg{q}",
                                       bufs=1, name=f"sagg{hd}")
                        eng = nc.vector if q % 2 == 0 else nc.gpsimd
                        eng.tensor_copy(sagg[:], paggs[hd][:])
                        nc.gpsimd.dma_start(
                            szall[hd:hd + 1, :], sagg[NHID:NHID + 1, :])
                        saggs[hd] = sagg

                def evac_norm(sweep):
                    """Reciprocal + broadcast + normalized xcat blocks."""
                    rzf = Pc.tile([8, R], F32R, tag="rzf", bufs=2,
                                  name=f"rzf{sweep}")
                    rzs = Pc.tile([4, R], F32, tag="rzs", bufs=2)
                    nc.vector.memset(rzf[:].bitcast(F32), 0.0)
                    nc.vector.reciprocal_approx_accurate(
                        rzf[4 * sweep:4 * sweep + 4, :].bitcast(F32),
                        szall[4 * sweep:4 * sweep + 4, :], rzs[:])
                    for q, hd in enumerate(range(sweep * 4, sweep * 4 + 4)):
                        zb = PsS.tile([64, R], F32, tag="ps_s", bufs=2,
                                      name=f"zb{hd}")
                        nc.tensor.matmul(
                            zb[:], sel[:, hd * 64:(hd + 1) * 64], rzf[:],
                            start=True, stop=True)
                        eng = nc.vector if q % 2 == 0 else nc.gpsimd
                        eng.scalar_tensor_tensor(
                            sxc[:, hd, :], saggs[hd][0:NHID, :], 1.0, zb[:],
                            op0=ALU.bypass, op1=ALU.mult)

                def project_half(sweep):
                    """pH partial for this sweep's 4 heads -> dblk[sweep]."""
                    for ic in range(4):
                        pH = PsA.tile([128, 130], F32, tag="ps_a", bufs=2,
                                      name=f"pH{sweep}_{ic}")
                        for k, hd in enumerate(
                                range(sweep * 4, sweep * 4 + 4)):
                            nc.tensor.matmul(
                                pH[:], sxc[:, hd, ic * 128:(ic + 1) * 128],
                                sWoutP[:, hd, :],
                                start=(k == 0), stop=(k == 3))
                        sh2b = Pp.tile([128, 130], BF16, tag="sh2b", bufs=2)
                        nc.vector.tensor_copy(sh2b[:], pH[:])
                        nc.gpsimd.dma_start(
                            sfi2h[sweep][:, ic * 128:(ic + 1) * 128],
                            sh2b[:, 129:130])
                        nc.sync.dma_start(
                            dblk[sweep][ic * 128:(ic + 1) * 128, :],
                            sh2b[:])

                def gather_half(sweep):
                    nc.gpsimd.collective_compute(
                        "AllGather", ALU.bypass,
                        replica_groups=[list(range(NCORES))],
                        ins=[dblk[sweep][:].opt()],
                        outs=[dgath[sweep][:].opt()])

                em_cur = issue_em(0, 0)
                for sweep in range(2):
                    heads = list(range(sweep * 4, sweep * 4 + 4))
                    for pr in range(NP):
                        if pr + 1 < NP:
                            em_next = issue_em(sweep, pr + 1)
                        elif sweep == 0:
                            em_next = issue_em(1, 0)
                        P4e = Pc.tile([128, 2, 2048], BF16, tag="p4e",
                                      bufs=2)
                        P4m = Pc.tile([128, 2, 2048], BF16, tag="p4m",
                                      bufs=2)
                        nc.scalar.activation(
                            P4e[:].rearrange("p i c -> p (i c)"),
                            em_cur[:].rearrange("p i c -> p (i c)"), AF.Exp)
                        em_cur = em_next
                        if sweep == 0:
                            for dj in (4, 5):
                                if 2 * pr + dj < JC:
                                    prep_jc(2 * pr + dj)
                        for i in range(2):
                            jc = 2 * pr + i
                            for q, hd in enumerate(heads):
                                sl = slice(q * 512, (q + 1) * 512)
                                if i * 4 + q < 6:
                                    nc.vector.tensor_tensor(
                                        P4m[:, i, sl], P4e[:, i, sl],
                                        masks[:, jc, :], op=ALU.mult)
                                else:
                                    nc.gpsimd.scalar_tensor_tensor(
                                        P4m[:, i, sl], P4e[:, i, sl], 1.0,
                                        masks[:, jc, :],
                                        op0=ALU.bypass, op1=ALU.mult)
                        for i in range(2):
                            jc = 2 * pr + i
                            for q, hd in enumerate(heads):
                                if jc == 0:
                                    paggs[hd] = Pagg.tile(
                                        [NHID + 1, 512], F32,
                                        tag=f"agg{hd % 4}", bufs=1,
                                        name=f"agg_s{sweep}_{hd}")
                                nc.tensor.matmul(
                                    paggs[hd][:], shplus[:, jc, hd, :],
                                    P4m[:, i, q * 512:(q + 1) * 512],
                                    start=(jc == 0), stop=(jc == JC - 1))
                        # interleave sweep-0 epilogue into sweep 1 so the
                        # first half-gather hides under sweep-1 compute
                        if sweep == 1:
                            if pr == 0:
                                evac_stage(0)
                            elif pr == 1:
                                evac_norm(0)

                    if sweep == 1:
                        evac_stage(1)
                        evac_norm(1)
                        project_all()
                        nc.gpsimd.collective_compute(
                            "AllGather", ALU.bypass,
                            replica_groups=[list(range(NCORES))],
                            ins=[dblk2[:].opt()], outs=[dgath2[:].opt()])
                        # keep the PE hot across the exposed gather
                        warm_pe(320, "agg2", masks[:, 0, :])

            # ======================== layer 2 ========================
            with tc.tile_pool(name="stage2", bufs=1) as P2:
                # own -0.8*fi2 broadcast
                sfi2s = P2.tile([1, R], BF16, name="sfi2s")
                nc.vector.tensor_scalar_mul(sfi2s[:], sfi2r[:], -0.8)
                fib208 = P2.tile([128, R], BF16, name="fib208")
                nc.gpsimd.partition_broadcast(fib208[:], sfi2s[:])

                sh2r = P2.tile([128, JC, 130], BF16, name="sh2r")
                sfj2 = P2.tile([128, JC], F32, name="sfj2")
                sfj22 = P2.tile([128, JC], F32, name="sfj22")
                for hh in range(4):
                    hsl = slice(hh * 8, (hh + 1) * 8)
                    nc.sync.dma_start(
                        sh2r[:, hsl, :],
                        dgath2[hh * 1024:(hh + 1) * 1024, :]
                        .rearrange("(jc p) c -> p jc c", p=128))
                    nc.vector.tensor_copy(sfj2[:, hsl], sh2r[:, hsl, 128])
                    nc.vector.tensor_scalar_mul(
                        sfj22[:, hsl], sfj2[:, hsl], ALPHA)

                def issue_em2(g):
                    em8 = P2.tile([128, 8, 512], BF16, tag="em8b", bufs=2,
                                  name=f"em8b_{g}")
                    for q in range(8):
                        jc = g * 8 + q
                        nc.vector.tensor_scalar(
                            em8[:, q, :], fib208[:],
                            sfj22[:, jc:jc + 1],
                            sfj2[:, jc:jc + 1],
                            op0=ALU.add, op1=ALU.max)
                    return em8

                pagg2 = Pagg.tile([128, 512], F32, tag="agg0", bufs=1)
                pZ2 = Pagg.tile([1, 512], F32, tag="agg1", bufs=1)
                em2_cur = issue_em2(0)
                for g in range(4):
                    if g + 1 < 4:
                        em2_next = issue_em2(g + 1)
                    P4e = P2.tile([128, 8, 512], BF16, tag="p4eb", bufs=2)
                    P4m = P2.tile([128, 8, 512], BF16, tag="p4mb", bufs=2)
                    nc.scalar.activation(
                        P4e[:].rearrange("p i c -> p (i c)"),
                        em2_cur[:].rearrange("p i c -> p (i c)"), AF.Exp)
                    em2_cur = em2_next
                    for q in range(8):
                        jc = g * 8 + q
                        nc.vector.tensor_tensor(
                            P4m[:, q, :], P4e[:, q, :], masks[:, jc, :],
                            op=ALU.mult)
                    for q in range(8):
                        jc = g * 8 + q
                        nc.tensor.matmul(
                            pZ2[:], onescol[:], P4m[:, q, :],
                            start=(jc == 0), stop=(jc == JC - 1))
                        nc.tensor.matmul(
                            pagg2[:], sh2r[:, jc, 0:128], P4m[:, q, :],
                            start=(jc == 0), stop=(jc == JC - 1))

                # normalize, elu, transpose, log-softmax
                srz2 = P2.tile([1, R], F32, name="srz2")
                srz2s = P2.tile([1, R], F32, name="srz2s")
                nc.vector.reciprocal_approx_accurate(
                    srz2[:], pZ2[0:1, :], srz2s[:])
                srz2b = P2.tile([1, R], BF16, name="srz2b")
                nc.vector.tensor_copy(srz2b[:], srz2[:])
                zb2 = PsS.tile([128, R], F32, tag="ps_s", bufs=2,
                               name="zb2")
                nc.tensor.matmul(zb2[:], ones1[:], srz2b[:],
                                 start=True, stop=True)
                zb2s = P2.tile([128, R], F32, name="zb2s")
                nc.vector.tensor_copy(zb2s[:], zb2[:])
                sv = P2.tile([128, R], F32, name="sv")
                nc.vector.scalar_tensor_tensor(
                    sv[:], pagg2[:], 1.0, zb2s[:],
                    op0=ALU.bypass, op1=ALU.mult)
                smin = P2.tile([128, R], F32, name="smin")
                nc.vector.tensor_scalar_min(smin[:], sv[:], 0.0)
                sex = P2.tile([128, R], F32, name="sex")
                nc.scalar.activation(sex[:], smin[:], AF.Exp)
                srel = P2.tile([128, R], F32, name="srel")
                nc.vector.tensor_scalar_max(srel[:], sv[:], 0.0)
                sres = P2.tile([128, R], F32, name="sres")
                nc.vector.scalar_tensor_tensor(
                    sres[:], sex[:], -1.0, srel[:],
                    op0=ALU.add, op1=ALU.add)

                mx4 = P2.tile([128, 4], F32, name="mx4")
                negmx4 = P2.tile([128, 4], F32, name="negmx4")
                ssum4 = P2.tile([128, 4], F32, name="ssum4")
                sln4 = P2.tile([128, 4], F32, name="sln4")
                b24 = P2.tile([128, 4], F32, name="b24")
                st4 = P2.tile([128, 4, 128], F32, name="st4")
                sout4 = P2.tile([128, 4, 128], F32, name="sout4")
                for it in range(4):
                    ptp = PsS.tile([128, 128], F32, tag="ps_s", bufs=2,
                                   name=f"ptp{it}")
                    nc.tensor.transpose(
                        ptp[:], sres[:, it * 128:(it + 1) * 128],
                        ident[:])
                    nc.vector.tensor_copy(st4[:, it, :], ptp[:])
                nc.vector.tensor_reduce(
                    mx4[:], st4[:], axis=mybir.AxisListType.X, op=ALU.max)
                nc.vector.tensor_scalar_mul(negmx4[:], mx4[:], -1.0)
                for it in range(4):
                    sexp = P2.tile([128, 128], F32, tag="sexp", bufs=2,
                                   name=f"sexp{it}")
                    nc.scalar.activation(
                        sexp[:], st4[:, it, :], AF.Exp,
                        bias=negmx4[:, it:it + 1],
                        accum_out=ssum4[:, it:it + 1])
                nc.scalar.activation(sln4[:], ssum4[:], AF.Ln)
                nc.vector.tensor_sub(b24[:], negmx4[:], sln4[:])
                for it in range(4):
                    nc.scalar.activation(sout4[:, it, :], st4[:, it, :],
                                         AF.Identity,
                                         bias=b24[:, it:it + 1])
                    nc.sync.dma_start(
                        out_d.ap()[it * 128:(it + 1) * 128, :],
                        sout4[:, it, :])

    nc.finalize()
    return nc


def _get_nc():
    if "nc" not in _CACHE:
        _CACHE["nc"] = _build_nc()
    return _CACHE["nc"]


def kernel(**inputs):
    x = np.asarray(inputs["x"], dtype=np.float32)
    adj = np.asarray(inputs["adj"])
    W = np.asarray(inputs["W"], dtype=np.float32)
    a = np.asarray(inputs["a"], dtype=np.float32)
    W_out = np.asarray(inputs["W_out"], dtype=np.float32)
    a_out = np.asarray(inputs["a_out"], dtype=np.float32)

    bf = ml_dtypes.bfloat16
    xT = np.ascontiguousarray(x.T).astype(bf)
    Wcat = np.ascontiguousarray(
        W.transpose(1, 0, 2).reshape(NFEAT, 512))
    A12 = np.zeros((512, 16), np.float32)
    for hd in range(NHEADS):
        A12[hd * NHID:(hd + 1) * NHID, hd] = a[hd, NHID:]      # a2 -> fj
        A12[hd * NHID:(hd + 1) * NHID, 8 + hd] = a[hd, :NHID]  # a1 -> fi
    W12 = (Wcat @ A12).astype(bf)
    AO = np.stack([a_out[NCLASS:], a_out[:NCLASS]], axis=1)    # [fj, fi]
    WoutP = np.concatenate([W_out, W_out @ AO], axis=1).astype(bf)
    ident = np.eye(128, dtype=np.float32)
    sel4 = np.zeros((4, 512), np.float32)
    for q in range(4):
        sel4[q, q * 64:(q + 1) * 64] = 1.0
    sel4 = sel4.astype(bf)
    adjf = adj.astype(np.float32)

    in_maps = []
    for c in range(NCORES):
        r0, r1 = c * R, (c + 1) * R
        in_maps.append({
            "xT": xT,
            "xTblk": np.ascontiguousarray(x[r0:r1].T).astype(bf),
            "Wcat": Wcat.astype(bf),
            "W12": W12,
            "WoutP": WoutP,
            "adjT": np.ascontiguousarray(adjf[r0:r1].T).astype(bf),
            "sel4": sel4,
            "ident": ident,
        })

    nc = _get_nc()
    trace = bool(os.environ.get("KERNEL_TRACE"))
    res = bass_utils.run_bass_kernel_spmd(
        nc, in_maps, list(range(NCORES)), trace=trace)
    kernel.last_results = res
    out = np.concatenate(
        [res.results[c]["out"] for c in range(NCORES)], axis=0)
    return np.ascontiguousarray(out, dtype=np.float32)


# revision 61
# speedup vs baseline: 1.6085x; 1.0021x over previous
"""GAT (2-layer multi-head graph attention) on 8 Trainium2 NeuronCores.

Sharding: nodes (rows of adj / attention) are sharded across the 8 cores;
each core computes h = x@W replicated, its 512-row block of
e/softmax/aggregation for both GAT layers.  The layer-1 -> layer-2
projections (h2|fj2|fi2 packed as 130 cols) are AllGathered in TWO halves:
the heads-0..3 partial right after sweep 0 (hidden under sweep 1), the
heads-4..7 partial at the end; layer 2 sums the gathered partials.

Layout: attention probabilities are computed TRANSPOSED (eT[j, i]) so the
softmax-normalizer and aggregation both run on the tensor engine via an
hplus = [h | 1] stationary operand (last row of the accumulator is Z).

Key factorization: softmax over j is invariant to any per-i shift, so we
compute e'[j,i] = leakyrelu(fi[i]+fj[j]) - fi[i] = max(fj, 0.2*fj - 0.8*fi).
With fib08 = -0.8*fi broadcast tiles precomputed once per head, the whole
e-map is ONE tensor_scalar (DVE 4x mode) per tile plus exp plus mask:
  em = (fib08 + 0.2*fj) max fj  (tensor_scalar, two ptr scalars, DVE 4x)
  P  = exp(em)                  (scalar engine; one [128,4096] op per 2 jc)
  Pm = P * adj                  (tensor_tensor 2x, split 6:2 DVE/Pool)
The 0/1 adjacency multiply replaces the -1e9 mask (exact zeros).
Weight-only transforms (Wcat, w12 = Wcat @ A12, w2 appended to Wout) are
folded on the host.  Z-reciprocal broadcasts are built with one-hot outer
products on the tensor engine.  Idle-window dummy matmuls keep the PE
p-state warm across the exposed collective.
"""
import os
import sys

for _p in ("/opt/trn_rl_repo", "/root/.axon_site/_ro/trn_rl_repo"):
    if os.path.isdir(_p) and _p not in sys.path:
        sys.path.insert(0, _p)

import numpy as np
import ml_dtypes

import concourse.bacc as bacc
import concourse.mybir as mybir
import concourse.tile as tile
from concourse import bass_utils

F32 = mybir.dt.float32
F32R = mybir.dt.float32r
BF16 = mybir.dt.bfloat16
AF = mybir.ActivationFunctionType
ALU = mybir.AluOpType

N, NFEAT, NHID, NCLASS, NHEADS = 4096, 512, 64, 128, 8
NCORES = 8
R = N // NCORES          # 512 rows per core
FC = NFEAT // 128        # 4 feature chunks
JC = N // 128            # 32 j-chunks
NP = JC // 2             # 16 jc-pairs
NP_BUILD = int(os.environ.get('NP_BUILD', '0'))
ALPHA = 0.2

_CACHE = {}
SKIP_WARM = bool(os.environ.get('SKIP_WARM'))
SKIP_L2 = bool(os.environ.get('SKIP_L2'))
SKIP_EVAC_ACT = bool(os.environ.get('SKIP_EVAC_ACT'))


def _build_nc():
    nc = bacc.Bacc("TRN2", target_bir_lowering=False, debug=False,
                   num_devices=NCORES)

    xT_d = nc.dram_tensor("xT", [NFEAT, N], BF16, kind="ExternalInput")
    xTb_d = nc.dram_tensor("xTblk", [NFEAT, R], BF16, kind="ExternalInput")
    Wcat_d = nc.dram_tensor("Wcat", [NFEAT, 512], BF16, kind="ExternalInput")
    W12_d = nc.dram_tensor("W12", [NFEAT, 16], BF16, kind="ExternalInput")
    WoutP_d = nc.dram_tensor("WoutP", [512, 130], BF16, kind="ExternalInput")
    adj_d = nc.dram_tensor("adjT", [N, R], BF16, kind="ExternalInput")
    sel_d = nc.dram_tensor("sel4", [4, 512], BF16, kind="ExternalInput")
    id_d = nc.dram_tensor("ident", [128, 128], F32, kind="ExternalInput")
    out_d = nc.dram_tensor("out", [R, NCLASS], F32, kind="ExternalOutput")

    with tile.TileContext(nc, num_cores=NCORES) as tc:
        with (
            tc.tile_pool(name="persist", bufs=1) as Pp,
            tc.tile_pool(name="dram", bufs=1, space="DRAM") as Pd,
            tc.tile_pool(name="psA", bufs=2, space="PSUM") as PsA,
            tc.tile_pool(name="psS", bufs=2, space="PSUM") as PsS,
            tc.tile_pool(name="pagg", bufs=1, space="PSUM") as Pagg,
        ):
            # ---- persistent constants ----
            onescol = Pp.tile([128, 1], BF16, name="onescol")
            nc.vector.memset(onescol[:], 1.0)
            ones1 = Pp.tile([1, 128], BF16, name="ones1")
            nc.vector.memset(ones1[:], 1.0)
            sel = Pp.tile([4, 512], BF16, name="sel")
            nc.sync.dma_start(sel[:], sel_d.ap())
            wprol = Pagg.tile([1, 512], F32, tag="agg3", bufs=1,
                              name="wprol")
            for _ in range(16):
                nc.tensor.matmul(wprol[:], onescol[0:4, :], sel[:],
                                 start=True, stop=True)

            # ---- DMAs ordered so the L1 pipeline can start ASAP ----
            sWcatF = Pp.tile([128, FC, 512], BF16, name="sWcatF")
            nc.sync.dma_start(
                sWcatF[:],
                Wcat_d.ap().rearrange("(fc p) o -> p fc o", p=128))
            sw12 = Pp.tile([128, FC, 16], BF16, name="sw12")
            nc.sync.dma_start(
                sw12[:], W12_d.ap().rearrange("(fc p) o -> p fc o", p=128))
            sxTb = Pp.tile([128, FC, R], BF16, name="sxTb")
            nc.sync.dma_start(
                sxTb[:], xTb_d.ap().rearrange("(fc p) r -> p fc r", p=128))
            ident = Pp.tile([128, 128], F32R, name="ident")
            nc.sync.dma_start(ident[:], id_d.ap())

            # x and adjacency stream in j-order, interleaved
            sx = Pp.tile([128, FC, N], BF16, name="sx")
            masks = Pp.tile([128, JC, R], BF16, name="masks")
            for g in range(4):
                nc.sync.dma_start(
                    sx[:, :, g * 1024:(g + 1) * 1024],
                    xT_d.ap()[:, g * 1024:(g + 1) * 1024]
                    .rearrange("(fc p) j -> p fc j", p=128))
                nc.sync.dma_start(
                    masks[:, g * 8:(g + 1) * 8, :],
                    adj_d.ap()[g * 1024:(g + 1) * 1024, :]
                    .rearrange("(jc p) r -> p jc r", p=128))
            sWoutP = Pp.tile([64, NHEADS, 130], BF16, name="sWoutP")
            nc.sync.dma_start(
                sWoutP[:], WoutP_d.ap().rearrange("(hd p) c -> p hd c", p=64))

            sfjT = Pp.tile([128, JC, 8], F32, name="sfjT")
            sfjT2 = Pp.tile([128, JC, 8], F32, name="sfjT2")  # 0.2 * fj
            fib08 = Pp.tile([128, NHEADS * R], BF16, name="fib08")  # -0.8*fi
            sxc = Pp.tile([64, NHEADS, R], BF16, name="sxc")
            szall = Pp.tile([8, R], F32, name="szall")

            shplus = Pp.tile([128, JC, NHEADS, NHID + 1], BF16, name="shplus")
            nc.vector.memset(shplus[:, :, :, NHID], 1.0)

            def warm_pe(n, tag, shape_src):
                """Dummy matmuls that keep the PE p-state ramped while it
                would otherwise idle (in-order PE queue = placement)."""
                w = Pagg.tile([1, 512], F32, tag=tag, bufs=1, name=f"w{tag}")
                for _ in range(n):
                    nc.tensor.matmul(w[:], onescol[:], shape_src,
                                     start=True, stop=True)

            # ================= prep: h tiles / fj / fi =================
            def prep_jc(jc):
                xa = [sx[:, fc, jc * 128:(jc + 1) * 128] for fc in range(FC)]
                pA = PsA.tile([128, 512], F32, tag="ps_a", bufs=2,
                              name=f"pA{jc}")
                for fc in range(FC):
                    nc.tensor.matmul(
                        pA[:], xa[fc], sWcatF[:, fc, :],
                        start=(fc == 0), stop=(fc == 3))
                nc.gpsimd.tensor_copy(
                    shplus[:, jc, :, 0:NHID],
                    pA[:].rearrange("p (hd o) -> p hd o", o=NHID))
                pfj = PsS.tile([128, 8], F32, tag="ps_s", bufs=2,
                               name=f"pfj{jc}")
                for fc in range(FC):
                    nc.tensor.matmul(
                        pfj[:], xa[fc], sw12[:, fc, 0:8],
                        start=(fc == 0), stop=(fc == 3))
                nc.gpsimd.tensor_copy(sfjT[:, jc, :], pfj[:])
                nc.gpsimd.tensor_scalar_mul(sfjT2[:, jc, :], pfj[:], ALPHA)

            with tc.tile_pool(name="stage1", bufs=1) as P1:
                # own-block fi scaled by -0.8, broadcast across partitions
                pfo = PsS.tile([16, R], F32, tag="ps_s", bufs=2)
                for fc in range(FC):
                    nc.tensor.matmul(
                        pfo[:], sw12[:, fc, :], sxTb[:, fc, :],
                        start=(fc == 0), stop=(fc == 3))
                sfob = P1.tile([16, R], BF16, name="sfob")
                nc.vector.tensor_scalar_mul(sfob[:], pfo[:], -0.8)
                fcat = P1.tile([1, NHEADS * R], BF16, name="fcat")
                nc.sync.dma_start(fcat[:], sfob[8:16, :])
                nc.gpsimd.partition_broadcast(fib08[:], fcat[:])
                prep_jc(0)
                prep_jc(1)
                prep_jc(2)
                prep_jc(3)

            # DRAM staging for the gather
            dblk2 = Pd.tile([R, 130], BF16, name="dblk2")
            dgath2 = Pd.tile([N, 130], BF16, name="dgath2",
                             addr_space="Shared")
            sfi2r = Pp.tile([1, R], BF16, name="sfi2r")

            # ================= layer-1 attention sweeps =================
            with tc.tile_pool(name="chunkL1", bufs=1) as Pc:
                def issue_em(sweep, pr):
                    em8 = Pc.tile([128, 2, 2048], BF16, tag="em8", bufs=3,
                                  name=f"em8_{sweep}_{pr}")
                    for i in range(2):
                        jc = 2 * pr + i
                        for q, hd in enumerate(
                                range(sweep * 4, sweep * 4 + 4)):
                            nc.vector.tensor_scalar(
                                em8[:, i, q * 512:(q + 1) * 512],
                                fib08[:, hd * R:(hd + 1) * R],
                                sfjT2[:, jc, hd:hd + 1],
                                sfjT[:, jc, hd:hd + 1],
                                op0=ALU.add, op1=ALU.max)
                    return em8

                paggs = {}
                saggs = {}

                def evac_stage(sweep):
                    """Stage sweep aggregates out of PSUM (frees the banks),
                    extract Z rows."""
                    for q, hd in enumerate(range(sweep * 4, sweep * 4 + 4)):
                        sagg = Pc.tile([NHID + 1, R], F32, tag=f"sagg{q}",
                                       bufs=1, name=f"sagg{hd}")
                        eng = nc.vector if q % 2 == 0 else nc.gpsimd
                        eng.tensor_copy(sagg[:], paggs[hd][:])
                        nc.gpsimd.dma_start(
                            szall[hd:hd + 1, :], sagg[NHID:NHID + 1, :])
                        saggs[hd] = sagg

                def evac_norm(sweep):
                    """Reciprocal + broadcast + normalized xcat blocks."""
                    rzf = Pc.tile([8, R], F32R, tag="rzf", bufs=2,
                                  name=f"rzf{sweep}")
                    rzs = Pc.tile([4, R], F32, tag="rzs", bufs=2)
                    nc.vector.memset(rzf[:].bitcast(F32), 0.0)
                    nc.vector.reciprocal_approx_accurate(
                        rzf[4 * sweep:4 * sweep + 4, :].bitcast(F32),
                        szall[4 * sweep:4 * sweep + 4, :], rzs[:])
                    for q, hd in enumerate(range(sweep * 4, sweep * 4 + 4)):
                        zb = PsS.tile([64, R], F32, tag="ps_s", bufs=2,
                                      name=f"zb{hd}")
                        nc.tensor.matmul(
                            zb[:], sel[:, hd * 64:(hd + 1) * 64], rzf[:],
                            start=True, stop=True)
                        eng = nc.vector if q % 2 == 0 else nc.gpsimd
                        eng.scalar_tensor_tensor(
                            sxc[:, hd, :], saggs[hd][0:NHID, :], 1.0, zb[:],
                            op0=ALU.bypass, op1=ALU.mult)

                def project_half(sweep):
                    """pH partial for this sweep's 4 heads -> dblk[sweep]."""
                    for ic in range(4):
                        pH = PsA.tile([128, 130], F32, tag="ps_a", bufs=2,
                                      name=f"pH{sweep}_{ic}")
                        for k, hd in enumerate(
                                range(sweep * 4, sweep * 4 + 4)):
                            nc.tensor.matmul(
                                pH[:], sxc[:, hd, ic * 128:(ic + 1) * 128],
                                sWoutP[:, hd, :],
                                start=(k == 0), stop=(k == 3))
                        sh2b = Pp.tile([128, 130], BF16, tag="sh2b", bufs=2)
                        nc.vector.tensor_copy(sh2b[:], pH[:])
                        nc.gpsimd.dma_start(
                            sfi2h[sweep][:, ic * 128:(ic + 1) * 128],
                            sh2b[:, 129:130])
                        nc.sync.dma_start(
                            dblk[sweep][ic * 128:(ic + 1) * 128, :],
                            sh2b[:])

                def gather_half(sweep):
                    nc.gpsimd.collective_compute(
                        "AllGather", ALU.bypass,
                        replica_groups=[list(range(NCORES))],
                        ins=[dblk[sweep][:].opt()],
                        outs=[dgath[sweep][:].opt()])

                em_cur = issue_em(0, 0)
                for sweep in range(2):
                    heads = list(range(sweep * 4, sweep * 4 + 4))
                    for pr in range(NP):
                        if pr + 1 < NP:
                            em_next = issue_em(sweep, pr + 1)
                        elif sweep == 0:
                            em_next = issue_em(1, 0)
                        P4e = Pc.tile([128, 2, 2048], BF16, tag="p4e",
                                      bufs=2)
                        P4m = Pc.tile([128, 2, 2048], BF16, tag="p4m",
                                      bufs=2)
                        nc.scalar.activation(
                            P4e[:].rearrange("p i c -> p (i c)"),
                            em_cur[:].rearrange("p i c -> p (i c)"), AF.Exp)
                        em_cur = em_next
                        if sweep == 0:
                            for dj in (4, 5):
                                if 2 * pr + dj < JC:
                                    prep_jc(2 * pr + dj)
                        for i in range(2):
                            jc = 2 * pr + i
                            for q, hd in enumerate(heads):
                                sl = slice(q * 512, (q + 1) * 512)
                                if i * 4 + q < 6:
                                    nc.vector.tensor_tensor(
                                        P4m[:, i, sl], P4e[:, i, sl],
                                        masks[:, jc, :], op=ALU.mult)
                                else:
                                    nc.gpsimd.scalar_tensor_tensor(
                                        P4m[:, i, sl], P4e[:, i, sl], 1.0,
                                        masks[:, jc, :],
                                        op0=ALU.bypass, op1=ALU.mult)
                        for i in range(2):
                            jc = 2 * pr + i
                            for q, hd in enumerate(heads):
                                if jc == 0:
                                    paggs[hd] = Pagg.tile(
                                        [NHID + 1, 512], F32,
                                        tag=f"agg{hd % 4}", bufs=1,
                                        name=f"agg_s{sweep}_{hd}")
                                nc.tensor.matmul(
                                    paggs[hd][:], shplus[:, jc, hd, :],
                                    P4m[:, i, q * 512:(q + 1) * 512],
                                    start=(jc == 0), stop=(jc == JC - 1))
                        # interleave sweep-0 epilogue into sweep 1 so the
                        # first half-gather hides under sweep-1 compute
                        if sweep == 1:
                            if pr == 0:
                                evac_stage(0)
                            elif pr == 1:
                                evac_norm(0)

                    if sweep == 1:
                        evac_stage(1)
                        evac_norm(1)
                        project_all()
                        nc.gpsimd.collective_compute(
                            "AllGather", ALU.bypass,
                            replica_groups=[list(range(NCORES))],
                            ins=[dblk2[:].opt()], outs=[dgath2[:].opt()])
                        # keep the PE hot across the exposed gather
                        warm_pe(320, "agg2", masks[:, 0, :])

            # ======================== layer 2 ========================
            with tc.tile_pool(name="stage2", bufs=1) as P2:
                # own -0.8*fi2 broadcast
                sfi2s = P2.tile([1, R], BF16, name="sfi2s")
                nc.vector.tensor_scalar_mul(sfi2s[:], sfi2r[:], -0.8)
                fib208 = P2.tile([128, R], BF16, name="fib208")
                nc.gpsimd.partition_broadcast(fib208[:], sfi2s[:])

                sh2r = P2.tile([128, JC, 130], BF16, name="sh2r")
                sfj2 = P2.tile([128, JC], F32, name="sfj2")
                sfj22 = P2.tile([128, JC], F32, name="sfj22")
                for hh in range(4):
                    hsl = slice(hh * 8, (hh + 1) * 8)
                    nc.sync.dma_start(
                        sh2r[:, hsl, :],
                        dgath2[hh * 1024:(hh + 1) * 1024, :]
                        .rearrange("(jc p) c -> p jc c", p=128))
                    nc.vector.tensor_copy(sfj2[:, hsl], sh2r[:, hsl, 128])
                    nc.vector.tensor_scalar_mul(
                        sfj22[:, hsl], sfj2[:, hsl], ALPHA)

                def issue_em2(g):
                    em8 = P2.tile([128, 8, 512], BF16, tag="em8b", bufs=2,
                                  name=f"em8b_{g}")
                    for q in range(8):
                        jc = g * 8 + q
                        nc.vector.tensor_scalar(
                            em8[:, q, :], fib208[:],
                            sfj22[:, jc:jc + 1],
                            sfj2[:, jc:jc + 1],
                            op0=ALU.add, op1=ALU.max)
                    return em8

                pagg2 = Pagg.tile([128, 512], F32, tag="agg0", bufs=1)
                pZ2 = Pagg.tile([1, 512], F32, tag="agg1", bufs=1)
                em2_cur = issue_em2(0)
                for g in range(4):
                    if g + 1 < 4:
                        em2_next = issue_em2(g + 1)
                    P4e = P2.tile([128, 8, 512], BF16, tag="p4eb", bufs=2)
                    P4m = P2.tile([128, 8, 512], BF16, tag="p4mb", bufs=2)
                    nc.scalar.activation(
                        P4e[:].rearrange("p i c -> p (i c)"),
                        em2_cur[:].rearrange("p i c -> p (i c)"), AF.Exp)
                    em2_cur = em2_next
                    for q in range(8):
                        jc = g * 8 + q
                        nc.vector.tensor_tensor(
                            P4m[:, q, :], P4e[:, q, :], masks[:, jc, :],
                            op=ALU.mult)
                    for q in range(8):
                        jc = g * 8 + q
                        nc.tensor.matmul(
                            pZ2[:], onescol[:], P4m[:, q, :],
                            start=(jc == 0), stop=(jc == JC - 1))
                        nc.tensor.matmul(
                            pagg2[:], sh2r[:, jc, 0:128], P4m[:, q, :],
                            start=(jc == 0), stop=(jc == JC - 1))

                # normalize, elu, transpose, log-softmax
                srz2 = P2.tile([1, R], F32, name="srz2")
                srz2s = P2.tile([1, R], F32, name="srz2s")
                nc.vector.reciprocal_approx_accurate(
                    srz2[:], pZ2[0:1, :], srz2s[:])
                srz2b = P2.tile([1, R], BF16, name="srz2b")
                nc.vector.tensor_copy(srz2b[:], srz2[:])
                zb2 = PsS.tile([128, R], F32, tag="ps_s", bufs=2,
                               name="zb2")
                nc.tensor.matmul(zb2[:], ones1[:], srz2b[:],
                                 start=True, stop=True)
                zb2s = P2.tile([128, R], F32, name="zb2s")
                nc.vector.tensor_copy(zb2s[:], zb2[:])
                sv = P2.tile([128, R], F32, name="sv")
                nc.vector.scalar_tensor_tensor(
                    sv[:], pagg2[:], 1.0, zb2s[:],
                    op0=ALU.bypass, op1=ALU.mult)
                smin = P2.tile([128, R], F32, name="smin")
                nc.vector.tensor_scalar_min(smin[:], sv[:], 0.0)
                sex = P2.tile([128, R], F32, name="sex")
                nc.scalar.activation(sex[:], smin[:], AF.Exp)
                srel = P2.tile([128, R], F32, name="srel")
                nc.vector.tensor_scalar_max(srel[:], sv[:], 0.0)
                sres = P2.tile([128, R], F32, name="sres")
                nc.vector.scalar_tensor_tensor(
                    sres[:], sex[:], -1.0, srel[:],
                    op0=ALU.add, op1=ALU.add)

                mx4 = P2.tile([128, 4], F32, name="mx4")
                negmx4 = P2.tile([128, 4], F32, name="negmx4")
                ssum4 = P2.tile([128, 4], F32, name="ssum4")
                sln4 = P2.tile([128, 4], F32, name="sln4")
                b24 = P2.tile([128, 4], F32, name="b24")
                st4 = P2.tile([128, 4, 128], F32, name="st4")
                sout4 = P2.tile([128, 4, 128], F32, name="sout4")
                for it in range(4):
                    ptp = PsS.tile([128, 128], F32, tag="ps_s", bufs=2,
                                   name=f"ptp{it}")
                    nc.tensor.transpose(
                        ptp[:], sres[:, it * 128:(it + 1) * 128],
                        ident[:])
                    nc.vector.tensor_copy(st4[:, it, :], ptp[:])
                nc.vector.tensor_reduce(
                    mx4[:], st4[:], axis=mybir.AxisListType.X, op=ALU.max)
                nc.vector.tensor_scalar_mul(negmx4[:], mx4[:], -1.0)
                for it in range(4):
                    sexp = P2.tile([128, 128], F32, tag="sexp", bufs=2,
                                   name=f"sexp{it}")
                    nc.scalar.activation(
                        sexp[:], st4[:, it, :], AF.Exp,
                        bias=negmx4[:, it:it + 1],
                        accum_out=ssum4[:, it:it + 1])
                nc.scalar.activation(sln4[:], ssum4[:], AF.Ln)
                nc.vector.tensor_sub(b24[:], negmx4[:], sln4[:])
                for it in range(4):
                    nc.scalar.activation(sout4[:, it, :], st4[:, it, :],
                                         AF.Identity,
                                         bias=b24[:, it:it + 1])
                    nc.sync.dma_start(
                        out_d.ap()[it * 128:(it + 1) * 128, :],
                        sout4[:, it, :])

    nc.finalize()
    return nc


def _get_nc():
    if "nc" not in _CACHE:
        _CACHE["nc"] = _build_nc()
    return _CACHE["nc"]


def kernel(**inputs):
    x = np.asarray(inputs["x"], dtype=np.float32)
    adj = np.asarray(inputs["adj"])
    W = np.asarray(inputs["W"], dtype=np.float32)
    a = np.asarray(inputs["a"], dtype=np.float32)
    W_out = np.asarray(inputs["W_out"], dtype=np.float32)
    a_out = np.asarray(inputs["a_out"], dtype=np.float32)

    bf = ml_dtypes.bfloat16
    xT = np.ascontiguousarray(x.T).astype(bf)
    Wcat = np.ascontiguousarray(
        W.transpose(1, 0, 2).reshape(NFEAT, 512))
    A12 = np.zeros((512, 16), np.float32)
    for hd in range(NHEADS):
        A12[hd * NHID:(hd + 1) * NHID, hd] = a[hd, NHID:]      # a2 -> fj
        A12[hd * NHID:(hd + 1) * NHID, 8 + hd] = a[hd, :NHID]  # a1 -> fi
    W12 = (Wcat @ A12).astype(bf)
    AO = np.stack([a_out[NCLASS:], a_out[:NCLASS]], axis=1)    # [fj, fi]
    WoutP = np.concatenate([W_out, W_out @ AO], axis=1).astype(bf)
    ident = np.eye(128, dtype=np.float32)
    sel4 = np.zeros((4, 512), np.float32)
    for q in range(4):
        sel4[q, q * 64:(q + 1) * 64] = 1.0
    sel4 = sel4.astype(bf)
    adjf = adj.astype(np.float32)

    in_maps = []
    for c in range(NCORES):
        r0, r1 = c * R, (c + 1) * R
        in_maps.append({
            "xT": xT,
            "xTblk": np.ascontiguousarray(x[r0:r1].T).astype(bf),
            "Wcat": Wcat.astype(bf),
            "W12": W12,
            "WoutP": WoutP,
            "adjT": np.ascontiguousarray(adjf[r0:r1].T).astype(bf),
            "sel4": sel4,
            "ident": ident,
        })

    nc = _get_nc()
    trace = bool(os.environ.get("KERNEL_TRACE"))
    res = bass_utils.run_bass_kernel_spmd(
        nc, in_maps, list(range(NCORES)), trace=trace)
    kernel.last_results = res
    out = np.concatenate(
        [res.results[c]["out"] for c in range(NCORES)], axis=0)
    return np.ascontiguousarray(out, dtype=np.float32)
